# revision 1
# baseline (speedup 1.0000x reference)
"""MipNerf IPE encoding kernel for Trainium2 (Bass/Tile), 8-core SPMD.

Computes reference(ray_o, ray_d, fg_z_vals, bg_z_vals, radii) -> [2048, 64, 768]:
  fg: diagonal-cov cone cast + diagonal IPE (48 sin + 48 cos features)
  bg: full-cov cone cast + contraction Jacobian + icosahedral-basis IPE
      (336 sin + 336 cos features)

Sharding: embarrassingly data-parallel over rays; 256 rays per core.

v2 pipeline (per 128-ray tile; 8-sample "eighths" stream through):
  - algebra -> yvb (variances, f32 [s*24+f]) and u0 (int32 fixed-point
    angle fractions, u0 = round(frac(y0/2pi) * 2^32))
  - E_half  f16[j*768+s*24+f] = exp(-0.5 * 4^j * yvb): 16 ACT instrs/half
  - Usin_e  i32[j*192+s*24+f] = u0 << j via log-step shift cascade
    (copy, <<1, <<2, <<4, <<8 on doubling block sizes) -- DVE int 2x
  - Ucos_e  f16 = |f16(Usin_e * 2^-32)| (one TS mult i32->f16 + one
    AND 0x7FFF at 4x) -- the wrapped angle magnitude in turns
  - S_e = Sin2pi(2^-32 * Usin_e) (ACT reads i32 directly, f16 out)
    C_e = Sin2pi(-Ucos_e + 0.25) (cos via phase flip, f16 out)
  - out_e f32[s*768+col] = S/C * E via 4 strided tensor_tensor mults
    (f16 x f16 -> f32), split across DVE and GpSimd; DMA per eighth
"""

import numpy as np

import concourse.bass as bass
import concourse.tile as tile
from concourse import mybir

F32 = mybir.dt.float32
F16 = mybir.dt.float16
I32 = mybir.dt.int32
U32 = mybir.dt.uint32
U16 = mybir.dt.uint16
AF = mybir.ActivationFunctionType
OP = mybir.AluOpType

MAGIC_RND = 12582912.0          # 1.5 * 2^23, float32 round-to-nearest trick
RSQRT_MAGIC = 0x5F3759DF
INV2PI = float(1.0 / (2.0 * np.pi))
TINY = 1e-6

# icosahedral basis (matches reference.py)
P_BASIS = np.array([
    0.8506508, 0.0, 0.5257311, 0.809017, 0.5, 0.309017, 0.5257311, 0.8506508, 0.0,
    1.0, 0.0, 0.0, 0.809017, 0.5, -0.309017, 0.8506508, 0.0, -0.5257311, 0.309017,
    0.809017, -0.5, 0.0, 0.5257311, -0.8506508, 0.5, 0.309017, -0.809017, 0.0, 1.0,
    0.0, -0.5257311, 0.8506508, 0.0, -0.309017, 0.809017, -0.5, 0.0, 0.5257311,
    0.8506508, -0.309017, 0.809017, 0.5, 0.309017, 0.809017, 0.5, 0.5, 0.309017,
    0.809017, 0.5, -0.309017, 0.809017, 0.0, 0.0, 1.0, -0.5, 0.309017, 0.809017,
    -0.809017, 0.5, 0.309017, -0.809017, 0.5, -0.309017], dtype=np.float32).reshape(3, 21)

N_CORES = 8
RAYS_PER_CORE = 256
NS = 64           # samples per ray
NL = 16           # frequency levels
NF = 24           # 21 bg basis dims + 3 fg axes
HALF = 32         # samples per half-tile
EI = 8            # samples per eighth (output block)
FOUT = 768

# which eighths (by index 0..7 within tile) run their bg_sin mult on DVE
# (the rest go to GpSimd) -- load-balancing knob
DVE_BGSIN = {0, 1, 2, 3, 4, 5, 6, 7}


# ---------------------------------------------------------------------------
# walrus workarounds
# ---------------------------------------------------------------------------

_PATCHED = False


def _apply_patches():
    """1) split >1 sem-waits per instruction (this walrus rejects multi-wait
    instructions);  2) rewrite sentinel Arctan activations into Sin2pi."""
    global _PATCHED
    if _PATCHED:
        return
    _PATCHED = True

    import concourse.bass2jax as bass2jax

    orig_compile = bass2jax.compile_bir_kernel

    def patched_compile(bir_json, tmpdir, neff_name="file.neff"):
        if isinstance(bir_json, bytes):
            bir_json = bir_json.replace(b'"func":"Arctan"', b'"func":"Sin2pi"')
        else:
            bir_json = bir_json.replace('"func":"Arctan"', '"func":"Sin2pi"')
        return orig_compile(bir_json, tmpdir, neff_name=neff_name)

    bass2jax.compile_bir_kernel = patched_compile


_waitsplit_ctr = [0]


def _split_sync_waits(nc, max_waits=1):
    n_split = 0
    for fn in nc.m.functions:
        for bb in fn.blocks:
            il = bb.instructions
            i = 0
            while i < len(il):
                ins = il[i]
                si = ins.sync_info
                waits = list(si.on_wait) if si is not None else []
                if len(waits) > max_waits:
                    extra, keep = waits[:-max_waits], waits[-max_waits:]
                    pos = i
                    for j in range(0, len(extra), max_waits):
                        chunk = extra[j:j + max_waits]
                        _waitsplit_ctr[0] += 1
                        nop = mybir.InstNoOp(
                            name=f"waitsplit_{_waitsplit_ctr[0]}", ins=[], outs=[])
                        nop.engine = ins.engine
                        nop.sync_info = mybir.SyncInfo(on_wait=chunk, on_update=[])
                        nc.register_instruction(nop, overwrite=True)
                        il.insert(pos, nop)
                        pos += 1
                        i += 1
                    ins.sync_info = mybir.SyncInfo(
                        on_wait=keep, on_update=list(si.on_update))
                    n_split += 1
                i += 1
    return n_split


# ---------------------------------------------------------------------------
# AP helpers
# ---------------------------------------------------------------------------

def _ap(base, offset_elems, dims):
    """Custom AP over a tile/AP: keep partition dim, replace free dims."""
    return bass.AP(tensor=base.tensor, offset=base.offset + offset_elems,
                   ap=[base.ap[0]] + [list(d) for d in dims])


# ---------------------------------------------------------------------------
# kernel body
# ---------------------------------------------------------------------------

def _moments(nc, cols, wide, z, r2, out_tm2, out_tv, out_rv, n=NS):
    """Frustum moments -> t_mean2 (=2*t_mean), t_var, r_var [128, n].
    r2 = radii^2 per-ray [128, 1].  For the fused fg+bg call, z is a
    [128, 130] concat of both z_vals and t0/t1 are 2-dim APs; n=128."""
    if n == NS:
        t0 = z[:, 0:NS]
        t1 = z[:, 1:NS + 1]
    else:
        t0 = _ap(z[:], 0, [[NS + 1, 2], [1, NS]])
        t1 = _ap(z[:], 1, [[NS + 1, 2], [1, NS]])
    sm = wide.tile([128, n], F32, tag="mo_a")
    nc.vector.tensor_tensor(out=sm[:], in0=t0, in1=t1, op=OP.add)
    df = wide.tile([128, n], F32, tag="mo_b")
    nc.vector.tensor_tensor(out=df[:], in0=t1, in1=t0, op=OP.subtract)
    sm2 = wide.tile([128, n], F32, tag="mo_c")
    nc.vector.tensor_tensor(out=sm2[:], in0=sm[:], in1=sm[:], op=OP.mult)
    df2 = wide.tile([128, n], F32, tag="mo_d")
    nc.vector.tensor_tensor(out=df2[:], in0=df[:], in1=df[:], op=OP.mult)
    # denom4 = 3*sm2 + df2
    den4 = wide.tile([128, n], F32, tag="mo_e")
    nc.vector.scalar_tensor_tensor(out=den4[:], in0=sm2[:], scalar=3.0,
                                   in1=df2[:], op0=OP.mult, op1=OP.add)
    rden4 = wide.tile([128, n], F32, tag="mo_f")
    nc.vector.reciprocal(out=rden4[:], in_=den4[:])
    u1 = wide.tile([128, n], F32, tag="mo_g")
    nc.vector.tensor_tensor(out=u1[:], in0=df2[:], in1=rden4[:], op=OP.mult)
    # t_mean2 = sm * (1 + 2*u1)
    tmp = wide.tile([128, n], F32, tag="mo_h")
    nc.vector.tensor_scalar(out=tmp[:], in0=u1[:], scalar1=2.0, scalar2=1.0,
                            op0=OP.mult, op1=OP.add)
    nc.vector.tensor_tensor(out=out_tm2[:], in0=sm[:], in1=tmp[:], op=OP.mult)
    # t_var = df2/12 - (4/15) * u1^2 * (den4 - 1.25*df2)
    u1sq = wide.tile([128, n], F32, tag="mo_h")
    nc.vector.tensor_tensor(out=u1sq[:], in0=u1[:], in1=u1[:], op=OP.mult)
    g2 = wide.tile([128, n], F32, tag="mo_a")
    nc.vector.scalar_tensor_tensor(out=g2[:], in0=df2[:], scalar=-1.25,
                                   in1=den4[:], op0=OP.mult, op1=OP.add)
    g3 = wide.tile([128, n], F32, tag="mo_c")
    nc.vector.tensor_tensor(out=g3[:], in0=u1sq[:], in1=g2[:], op=OP.mult)
    g5 = wide.tile([128, n], F32, tag="mo_e")
    nc.vector.tensor_scalar_mul(out=g5[:], in0=df2[:], scalar1=float(1.0 / 12.0))
    nc.vector.scalar_tensor_tensor(out=out_tv[:], in0=g3[:], scalar=float(-4.0 / 15.0),
                                   in1=g5[:], op0=OP.mult, op1=OP.add)
    # r_var = r2 * (sm2/16 + (5/48)*df2 - (1/15)*u1*df2)
    h1 = wide.tile([128, n], F32, tag="mo_a")
    nc.vector.tensor_tensor(out=h1[:], in0=u1[:], in1=df2[:], op=OP.mult)
    h2 = wide.tile([128, n], F32, tag="mo_c")
    nc.vector.tensor_scalar_mul(out=h2[:], in0=sm2[:], scalar1=float(1.0 / 16.0))
    h4 = wide.tile([128, n], F32, tag="mo_e")
    nc.vector.scalar_tensor_tensor(out=h4[:], in0=df2[:], scalar=float(5.0 / 48.0),
                                   in1=h2[:], op0=OP.mult, op1=OP.add)
    h5 = wide.tile([128, n], F32, tag="mo_a")
    nc.vector.scalar_tensor_tensor(out=h5[:], in0=h1[:], scalar=float(-1.0 / 15.0),
                                   in1=h4[:], op0=OP.mult, op1=OP.add)
    nc.vector.tensor_scalar_mul(out=out_rv[:], in0=h5[:], scalar1=r2[:])


def build_kernel():
    """Build the 8-core SPMD Bass module (per-core: 256 rays)."""
    _apply_patches()
    nc = bass.Bass(dynamic_dma_scratch_size=4096)

    ray_o = nc.dram_tensor("ray_o", [RAYS_PER_CORE, 3], F32, kind="ExternalInput")
    ray_d = nc.dram_tensor("ray_d", [RAYS_PER_CORE, 3], F32, kind="ExternalInput")
    fg_z = nc.dram_tensor("fg_z", [RAYS_PER_CORE, NS + 1], F32, kind="ExternalInput")
    bg_z = nc.dram_tensor("bg_z", [RAYS_PER_CORE, NS + 1], F32, kind="ExternalInput")
    radii = nc.dram_tensor("radii", [RAYS_PER_CORE, 1], F32, kind="ExternalInput")
    pconst = nc.dram_tensor("pconst", [1, 84], F32, kind="ExternalInput")
    out = nc.dram_tensor("out", [RAYS_PER_CORE, NS * FOUT], F32, kind="ExternalOutput")

    with tile.TileContext(nc) as tc:
        import contextlib
        ctx = contextlib.ExitStack()
        with ctx:
            consts = ctx.enter_context(tc.tile_pool(name="consts", bufs=1))
            cols = ctx.enter_context(tc.tile_pool(name="cols", bufs=1))
            wide = ctx.enter_context(tc.tile_pool(name="wide", bufs=2))
            base = ctx.enter_context(tc.tile_pool(name="base", bufs=1))
            yvp = ctx.enter_context(tc.tile_pool(name="yvp", bufs=1))
            u0p = ctx.enter_context(tc.tile_pool(name="u0p", bufs=1))
            ep = ctx.enter_context(tc.tile_pool(name="ep", bufs=2))
            usp = ctx.enter_context(tc.tile_pool(name="usp", bufs=2))
            ucp = ctx.enter_context(tc.tile_pool(name="ucp", bufs=1))
            sp = ctx.enter_context(tc.tile_pool(name="sp", bufs=3))
            cp = ctx.enter_context(tc.tile_pool(name="cp", bufs=3))
            outp = ctx.enter_context(tc.tile_pool(name="outp", bufs=2))
            outq = ctx.enter_context(tc.tile_pool(name="outq", bufs=2))

            # constants
            pc = consts.tile([128, 84], F32)
            pca = pconst[:, :]
            nc.sync.dma_start(out=pc[:], in_=bass.AP(
                tensor=pca.tensor, offset=pca.offset, ap=[[0, 128], [1, 84]]))
            magic_u = consts.tile([128, 1], U32)
            nc.vector.memset(magic_u, RSQRT_MAGIC)
            quarter = consts.tile([128, 1], F32)
            nc.vector.memset(quarter, 0.25)

            yvbs, u0s = [], []
            for t in range(2):
                r0 = t * 128

                # ---------------- load inputs ----------------
                z2 = base.tile([128, 2 * (NS + 1)], F32, tag="z2")
                nc.sync.dma_start(out=z2[:, 0:NS + 1], in_=fg_z[r0:r0 + 128, :])
                nc.sync.dma_start(out=z2[:, NS + 1:2 * (NS + 1)],
                                  in_=bg_z[r0:r0 + 128, :])
                o3 = base.tile([128, 3], F32, tag="o3")
                nc.sync.dma_start(out=o3[:], in_=ray_o[r0:r0 + 128, :])
                d3 = base.tile([128, 3], F32, tag="d3")
                nc.sync.dma_start(out=d3[:], in_=ray_d[r0:r0 + 128, :])
                rad = base.tile([128, 1], F32, tag="rad")
                nc.sync.dma_start(out=rad[:], in_=radii[r0:r0 + 128, :])

                # ---------------- per-ray scalars ----------------
                r2 = cols.tile([128, 1], F32, tag="r2")
                nc.vector.tensor_tensor(out=r2[:], in0=rad[:], in1=rad[:], op=OP.mult)
                dk2 = cols.tile([128, 3], F32, tag="dk2")
                nc.vector.tensor_tensor(out=dk2[:], in0=d3[:], in1=d3[:], op=OP.mult)
                dmag = cols.tile([128, 1], F32, tag="dmag")
                nc.vector.tensor_tensor(out=dmag[:], in0=dk2[:, 0:1], in1=dk2[:, 1:2], op=OP.add)
                nc.vector.tensor_tensor(out=dmag[:], in0=dmag[:], in1=dk2[:, 2:3], op=OP.add)
                nc.vector.tensor_scalar_max(out=dmag[:], in0=dmag[:], scalar1=1e-8)
                rdmag = cols.tile([128, 1], F32, tag="rdmag")
                nc.vector.reciprocal(out=rdmag[:], in_=dmag[:])
                hd3 = cols.tile([128, 3], F32, tag="hd3")
                nc.vector.tensor_scalar_mul(out=hd3[:], in0=d3[:], scalar1=0.5)

                # e = d @ P  [128, 21], esq
                e21 = cols.tile([128, 21], F32, tag="e21")
                nc.vector.tensor_scalar_mul(out=e21[:], in0=pc[:, 0:21], scalar1=d3[:, 0:1])
                tmp21 = cols.tile([128, 21], F32, tag="tmp21")
                nc.vector.tensor_scalar_mul(out=tmp21[:], in0=pc[:, 21:42], scalar1=d3[:, 1:2])
                nc.vector.tensor_tensor(out=e21[:], in0=e21[:], in1=tmp21[:], op=OP.add)
                nc.vector.tensor_scalar_mul(out=tmp21[:], in0=pc[:, 42:63], scalar1=d3[:, 2:3])
                nc.vector.tensor_tensor(out=e21[:], in0=e21[:], in1=tmp21[:], op=OP.add)
                esq = cols.tile([128, 21], F32, tag="esq")
                nc.vector.tensor_tensor(out=esq[:], in0=e21[:], in1=e21[:], op=OP.mult)

                # ---------------- moments ----------------
                tm2a = cols.tile([128, 2 * NS], F32, tag="tm2a")
                tva = cols.tile([128, 2 * NS], F32, tag="tva")
                rva = cols.tile([128, 2 * NS], F32, tag="rva")
                _moments(nc, cols, wide, z2, r2, tm2a, tva, rva, n=2 * NS)
                tm2f, tm2b = tm2a[:, 0:NS], tm2a[:, NS:2 * NS]
                tvf, tvb = tva[:, 0:NS], tva[:, NS:2 * NS]
                rvf, rvb = rva[:, 0:NS], rva[:, NS:2 * NS]

                yb = base.tile([128, NF * NS], F32, tag="ybase")    # [s*24+f]
                yvb = yvp.tile([128, NF * NS], F32, tag=f"yv{t}")
                yvbs.append(yvb)

                # ---------------- fg: mean + cov_diag ----------------
                alf = wide.tile([128, NS], F32, tag="mo_b")
                nc.vector.tensor_scalar_mul(out=alf[:], in0=rvf, scalar1=rdmag[:])
                nc.vector.tensor_tensor(out=alf[:], in0=tvf, in1=alf[:], op=OP.subtract)
                for k in range(3):
                    # m_k = tm2f * halfd_k + o_k, written s-major at col 21+k
                    nc.vector.tensor_scalar(
                        out=_ap(yb[:], 21 + k, [[NF, NS]]), in0=tm2f,
                        scalar1=hd3[:, k:k + 1], scalar2=o3[:, k:k + 1],
                        op0=OP.mult, op1=OP.add)
                    # cd_k = alf * dk2_k + rvf
                    nc.vector.scalar_tensor_tensor(
                        out=_ap(yvb[:], 21 + k, [[NF, NS]]), in0=alf[:],
                        scalar=dk2[:, k:k + 1], in1=rvf, op0=OP.mult, op1=OP.add)

                # ---------------- bg: contraction scalars ----------------
                X = base.tile([128, 3 * NS], F32, tag="mk")          # [k*64+s]
                for k in range(3):
                    nc.vector.tensor_scalar(
                        out=X[:, k * NS:(k + 1) * NS], in0=tm2b,
                        scalar1=hd3[:, k:k + 1], scalar2=o3[:, k:k + 1],
                        op0=OP.mult, op1=OP.add)
                s2 = cols.tile([128, NS], F32, tag="s2")
                nc.vector.tensor_tensor(out=s2[:], in0=X[:, 0:NS], in1=X[:, 0:NS], op=OP.mult)
                w0 = wide.tile([128, NS], F32, tag="mo_a")
                nc.vector.tensor_tensor(out=w0[:], in0=X[:, NS:2 * NS], in1=X[:, NS:2 * NS], op=OP.mult)
                nc.vector.tensor_tensor(out=s2[:], in0=s2[:], in1=w0[:], op=OP.add)
                nc.vector.tensor_tensor(out=w0[:], in0=X[:, 2 * NS:3 * NS], in1=X[:, 2 * NS:3 * NS], op=OP.mult)
                nc.vector.tensor_tensor(out=s2[:], in0=s2[:], in1=w0[:], op=OP.add)
                # h = d . X
                h = cols.tile([128, NS], F32, tag="h")
                nc.vector.tensor_scalar_mul(out=h[:], in0=X[:, 0:NS], scalar1=d3[:, 0:1])
                nc.vector.scalar_tensor_tensor(out=h[:], in0=X[:, NS:2 * NS],
                                               scalar=d3[:, 1:2], in1=h[:],
                                               op0=OP.mult, op1=OP.add)
                nc.vector.scalar_tensor_tensor(out=h[:], in0=X[:, 2 * NS:3 * NS],
                                               scalar=d3[:, 2:3], in1=h[:],
                                               op0=OP.mult, op1=OP.add)

                # rsqrt(s2): magic seed + 4 Newton iterations
                rn0 = cols.tile([128, NS], F32, tag="rn0")
                seed_u = wide.tile([128, NS], U32, tag="mo_a")
                nc.vector.tensor_scalar(out=seed_u[:], in0=s2[:].bitcast(U32),
                                        scalar1=1, scalar2=None,
                                        op0=OP.logical_shift_right)
                nc.vector.tensor_tensor(
                    out=rn0[:].bitcast(U32),
                    in0=_ap(magic_u[:], 0, [[0, NS]]),
                    in1=seed_u[:], op=OP.subtract)
                for _ in range(4):
                    nr = wide.tile([128, NS], F32, tag="mo_b")
                    nc.vector.tensor_tensor(out=nr[:], in0=s2[:], in1=rn0[:], op=OP.mult)
                    nc.vector.tensor_tensor(out=nr[:], in0=nr[:], in1=rn0[:], op=OP.mult)
                    nc.vector.tensor_scalar(out=nr[:], in0=nr[:], scalar1=-0.5,
                                            scalar2=1.5, op0=OP.mult, op1=OP.add)
                    nc.vector.tensor_tensor(out=rn0[:], in0=rn0[:], in1=nr[:], op=OP.mult)

                n0 = cols.tile([128, NS], F32, tag="n0")
                nc.vector.tensor_tensor(out=n0[:], in0=s2[:], in1=rn0[:], op=OP.mult)
                rn = cols.tile([128, NS], F32, tag="rn")
                nc.vector.tensor_scalar(out=rn[:], in0=rn0[:], scalar1=-TINY,
                                        scalar2=1.0, op0=OP.mult, op1=OP.add)
                nc.vector.tensor_tensor(out=rn[:], in0=rn0[:], in1=rn[:], op=OP.mult)
                a_ = cols.tile([128, NS], F32, tag="a")
                nc.vector.tensor_scalar(out=a_[:], in0=rn[:], scalar1=-1.0,
                                        scalar2=2.0, op0=OP.mult, op1=OP.add)
                nc.vector.tensor_tensor(out=a_[:], in0=rn[:], in1=a_[:], op=OP.mult)
                b_ = cols.tile([128, NS], F32, tag="b")
                nc.vector.tensor_scalar_add(out=b_[:], in0=rn[:], scalar1=-1.0)
                t2_ = wide.tile([128, NS], F32, tag="mo_a")
                nc.vector.tensor_tensor(out=t2_[:], in0=rn[:], in1=rn0[:], op=OP.mult)
                nc.vector.tensor_tensor(out=t2_[:], in0=t2_[:], in1=rn[:], op=OP.mult)
                nc.vector.tensor_tensor(out=b_[:], in0=t2_[:], in1=b_[:], op=OP.mult)
                nc.vector.tensor_scalar_mul(out=b_[:], in0=b_[:], scalar1=2.0)

                # alpha_b, A coefficients
                alb = cols.tile([128, NS], F32, tag="alb")
                nc.vector.tensor_scalar_mul(out=alb[:], in0=rvb, scalar1=rdmag[:])
                nc.vector.tensor_tensor(out=alb[:], in0=tvb, in1=alb[:], op=OP.subtract)
                bh = cols.tile([128, NS], F32, tag="bh")
                nc.vector.tensor_tensor(out=bh[:], in0=b_[:], in1=h[:], op=OP.mult)
                asq = wide.tile([128, NS], F32, tag="mo_a")
                nc.vector.tensor_tensor(out=asq[:], in0=a_[:], in1=a_[:], op=OP.mult)
                A1 = cols.tile([128, NS], F32, tag="A1")
                nc.vector.tensor_tensor(out=A1[:], in0=alb[:], in1=asq[:], op=OP.mult)
                A4 = cols.tile([128, NS], F32, tag="A4")
                nc.vector.tensor_tensor(out=A4[:], in0=rvb, in1=asq[:], op=OP.mult)
                A2 = cols.tile([128, NS], F32, tag="A2")
                nc.vector.tensor_tensor(out=A2[:], in0=alb[:], in1=a_[:], op=OP.mult)
                nc.vector.tensor_tensor(out=A2[:], in0=A2[:], in1=bh[:], op=OP.mult)
                nc.vector.tensor_scalar_mul(out=A2[:], in0=A2[:], scalar1=2.0)
                # A3 = alb*bh^2 + rvb*(2ab + (b*n0)^2)
                A3 = cols.tile([128, NS], F32, tag="A3")
                bn = wide.tile([128, NS], F32, tag="mo_b")
                nc.vector.tensor_tensor(out=bn[:], in0=b_[:], in1=n0[:], op=OP.mult)
                nc.vector.tensor_tensor(out=bn[:], in0=bn[:], in1=bn[:], op=OP.mult)
                ab = wide.tile([128, NS], F32, tag="mo_c")
                nc.vector.tensor_tensor(out=ab[:], in0=a_[:], in1=b_[:], op=OP.mult)
                nc.vector.scalar_tensor_tensor(out=bn[:], in0=ab[:], scalar=2.0,
                                               in1=bn[:], op0=OP.mult, op1=OP.add)
                nc.vector.tensor_tensor(out=A3[:], in0=rvb, in1=bn[:], op=OP.mult)
                bh2 = wide.tile([128, NS], F32, tag="mo_a")
                nc.vector.tensor_tensor(out=bh2[:], in0=bh[:], in1=bh[:], op=OP.mult)
                nc.vector.tensor_tensor(out=bh2[:], in0=alb[:], in1=bh2[:], op=OP.mult)
                nc.vector.tensor_tensor(out=A3[:], in0=A3[:], in1=bh2[:], op=OP.add)

                # ---------------- c = X . p_q   [128, 21*64] ----------------
                c = base.tile([128, 21 * NS], F32, tag="c")
                w1 = base.tile([128, 21 * NS], F32, tag="w1")
                # c[p, q*64+s] = sum_k X[p, k*64+s] * P[k, q]
                GP0 = _ap(pc[:], 0, [[0, NS], [1, 21]])
                GP1 = _ap(pc[:], 21, [[0, NS], [1, 21]])
                GP2 = _ap(pc[:], 42, [[0, NS], [1, 21]])
                X0 = _ap(X[:], 0, [[1, NS], [0, 21]])
                X1 = _ap(X[:], NS, [[1, NS], [0, 21]])
                X2 = _ap(X[:], 2 * NS, [[1, NS], [0, 21]])
                nc.vector.tensor_tensor(out=c[:], in0=X0, in1=GP0, op=OP.mult)
                nc.vector.tensor_tensor(out=w1[:], in0=X1, in1=GP1, op=OP.mult)
                nc.vector.tensor_tensor(out=c[:], in0=c[:], in1=w1[:], op=OP.add)
                nc.vector.tensor_tensor(out=w1[:], in0=X2, in1=GP2, op=OP.mult)
                nc.vector.tensor_tensor(out=c[:], in0=c[:], in1=w1[:], op=OP.add)

                # ---------------- yv0 / y0 ----------------
                # yv0 = (A2*e + A3*c)*c + (A1*esq + A4*w)
                A2b = _ap(A2[:], 0, [[1, NS], [0, 21]])
                A3b = _ap(A3[:], 0, [[1, NS], [0, 21]])
                A1b = _ap(A1[:], 0, [[1, NS], [0, 21]])
                A4b = _ap(A4[:], 0, [[1, NS], [0, 21]])
                ab_ = _ap(a_[:], 0, [[1, NS], [0, 21]])
                e_b = _ap(e21[:], 0, [[0, NS], [1, 21]])
                esq_b = _ap(esq[:], 0, [[0, NS], [1, 21]])
                w_b = _ap(pc[:], 63, [[0, NS], [1, 21]])
                yvb_bg = _ap(yvb[:], 0, [[NF, NS], [1, 21]])
                nc.vector.tensor_tensor(out=yvb_bg, in0=A1b, in1=esq_b, op=OP.mult)
                nc.vector.tensor_tensor(out=w1[:], in0=A4b, in1=w_b, op=OP.mult)
                nc.vector.tensor_tensor(out=yvb_bg, in0=yvb_bg, in1=w1[:], op=OP.add)
                nc.vector.tensor_tensor(out=w1[:], in0=A2b, in1=e_b, op=OP.mult)
                nc.vector.tensor_tensor(out=w1[:], in0=w1[:], in1=c[:], op=OP.mult)
                nc.vector.tensor_tensor(out=yvb_bg, in0=yvb_bg, in1=w1[:], op=OP.add)
                nc.vector.tensor_tensor(out=w1[:], in0=A3b, in1=c[:], op=OP.mult)
                nc.vector.tensor_tensor(out=w1[:], in0=w1[:], in1=c[:], op=OP.mult)
                nc.vector.tensor_tensor(out=yvb_bg, in0=yvb_bg, in1=w1[:], op=OP.add)
                # y0 = a * c
                yb_bg = _ap(yb[:], 0, [[NF, NS], [1, 21]])
                nc.vector.tensor_tensor(out=yb_bg, in0=ab_, in1=c[:], op=OP.mult)

                # ---------------- angle -> int32 fraction ----------------
                # t = yb*inv2pi ; q = round(t) ; f0 = t - q ; u0 = f0 * 2^32
                tt = base.tile([128, NF * NS], F32, tag="w1")
                nc.vector.tensor_scalar(out=tt[:], in0=yb[:], scalar1=INV2PI,
                                        scalar2=MAGIC_RND, op0=OP.mult, op1=OP.add)
                nc.vector.tensor_scalar(out=tt[:], in0=tt[:], scalar1=MAGIC_RND,
                                        scalar2=None, op0=OP.subtract)
                nc.vector.scalar_tensor_tensor(out=yb[:], in0=yb[:], scalar=INV2PI,
                                               in1=tt[:], op0=OP.mult, op1=OP.subtract)
                f0 = yb
                u0 = u0p.tile([128, NF * NS], I32, tag=f"u0{t}")
                nc.vector.tensor_scalar_mul(out=u0[:], in0=f0[:], scalar1=float(2.0 ** 32))
                u0s.append(u0)

            # ---------------- streaming: interleaved tile streams ----------
            W = NF * EI                  # 192

            def angle_prep(t, e_idx):
                """Emit sin cascade + cos prep for one eighth (DVE)."""
                base_u = e_idx * NF * EI
                u0 = u0s[t]
                us = usp.tile([128, NL * W], I32, tag="us")
                nc.vector.tensor_copy(out=us[:, 0:W],
                                      in_=u0[:, base_u:base_u + W])
                for b, sh in ((1, 1), (2, 2), (4, 4), (8, 8)):
                    nc.vector.tensor_scalar(
                        out=us[:, b * W:2 * b * W], in0=us[:, 0:b * W],
                        scalar1=sh, scalar2=None,
                        op0=OP.logical_shift_left)
                uc = ucp.tile([128, NL * W], F16, tag="uc")
                nc.vector.tensor_scalar(out=uc[:], in0=us[:],
                                        scalar1=float(2.0 ** -32),
                                        scalar2=None, op0=OP.mult)
                nc.vector.tensor_scalar(out=uc[:].bitcast(U16),
                                        in0=uc[:].bitcast(U16),
                                        scalar1=0x7FFF, scalar2=None,
                                        op0=OP.bitwise_and)
                return us, uc

            def emit_exp(E, t, hh, j0, j1):
                for j in range(j0, j1):
                    nc.scalar.activation(
                        out=E[:, j * NF * HALF:(j + 1) * NF * HALF],
                        in_=yvbs[t][:, hh * NF * HALF:(hh + 1) * NF * HALF],
                        func=AF.Exp, scale=float(-0.5 * (4.0 ** j)))

            ang = {}
            ecur = ep.tile([128, NL * NF * HALF], F16, tag="E")
            emit_exp(ecur, 0, 0, 0, NL)     # prologue: E for (t=0, h=0)
            for t in range(2):
                ang[t] = angle_prep(t, 0)
                for hh in range(2):
                    nt, nh = (t, 1) if hh == 0 else (t + 1, 0)
                    enext = None
                    if nt < 2:
                        enext = ep.tile([128, NL * NF * HALF], F16, tag="E")

                    for ee in range(4):
                        e_idx = hh * 4 + ee          # eighth within tile
                        r0 = t * 128
                        E = ecur

                        # --- ACT: sin / cos values (f16) ---
                        us, uc = ang.pop(t)
                        S = sp.tile([128, NL * W], F16, tag="S")
                        nc.scalar.activation(out=S[:], in_=us[:], func=AF.Arctan,
                                             scale=float(2.0 ** -32))
                        C = cp.tile([128, NL * W], F16, tag="C")
                        nc.scalar.activation(out=C[:], in_=uc[:], func=AF.Arctan,
                                             scale=-1.0, bias=quarter[:])

                        # --- software pipeline: next eighth's angle prep ---
                        if e_idx < 7:
                            ang[t] = angle_prep(t, e_idx + 1)

                        # --- spread next half's exp instrs (4 per eighth) ---
                        if enext is not None:
                            emit_exp(enext, nt, nh, 4 * ee, 4 * ee + 4)

                        # --- final mults, split out buffers ---
                        # obA: cols 0..431 (fg_sin, fg_cos, bg_sin)
                        # obB: cols 432..767 (bg_cos)
                        obA = outp.tile([128, EI * 432], F32, tag="obA")
                        obB = outq.tile([128, EI * 336], F32, tag="obB")
                        e_off = ee * NF * EI   # offset into E for this eighth
                        o_bg_sin = _ap(obA[:], 96, [[432, EI], [21, NL], [1, 21]])
                        s_bg = _ap(S[:], 0, [[NF, EI], [W, NL], [1, 21]])
                        e_bg = _ap(E[:], e_off, [[NF, EI], [NF * HALF, NL], [1, 21]])
                        if e_idx in DVE_BGSIN:
                            nc.vector.tensor_tensor(out=o_bg_sin, in0=s_bg,
                                                    in1=e_bg, op=OP.mult)
                        else:
                            nc.gpsimd.tensor_tensor(out=o_bg_sin, in0=s_bg,
                                                    in1=e_bg, op=OP.mult)
                        o_bg_cos = _ap(obB[:], 0, [[336, EI], [21, NL], [1, 21]])
                        c_bg = _ap(C[:], 0, [[NF, EI], [W, NL], [1, 21]])
                        nc.gpsimd.tensor_tensor(out=o_bg_cos, in0=c_bg,
                                                in1=e_bg, op=OP.mult)
                        o_fg_sin = _ap(obA[:], 0, [[432, EI], [3, NL], [1, 3]])
                        s_fg = _ap(S[:], 21, [[NF, EI], [W, NL], [1, 3]])
                        e_fg = _ap(E[:], e_off + 21, [[NF, EI], [NF * HALF, NL], [1, 3]])
                        o_fg_cos = _ap(obA[:], 48, [[432, EI], [3, NL], [1, 3]])
                        c_fg = _ap(C[:], 21, [[NF, EI], [W, NL], [1, 3]])
                        fg_eng = nc.vector
                        fg_eng.tensor_tensor(out=o_fg_sin, in0=s_fg,
                                             in1=e_fg, op=OP.mult)
                        fg_eng.tensor_tensor(out=o_fg_cos, in0=c_fg,
                                             in1=e_fg, op=OP.mult)

                        # --- DMA out (two blocks) ---
                        oa = out[:, :]
                        nc.sync.dma_start(
                            out=bass.AP(
                                tensor=oa.tensor,
                                offset=oa.offset + r0 * NS * FOUT + e_idx * EI * FOUT,
                                ap=[[NS * FOUT, 128], [FOUT, EI], [1, 432]]),
                            in_=obA[:])
                        nc.sync.dma_start(
                            out=bass.AP(
                                tensor=oa.tensor,
                                offset=oa.offset + r0 * NS * FOUT + e_idx * EI * FOUT + 432,
                                ap=[[NS * FOUT, 128], [FOUT, EI], [1, 336]]),
                            in_=obB[:])
                    ecur = enext

    _split_sync_waits(nc)
    return nc


# ---------------------------------------------------------------------------
# entry point
# ---------------------------------------------------------------------------

_NC_CACHE = []


def kernel(ray_o, ray_d, fg_z_vals, bg_z_vals, radii):
    from concourse.bass_utils import run_bass_kernel_spmd

    if not _NC_CACHE:
        _NC_CACHE.append(build_kernel())
    nc = _NC_CACHE[0]

    pconst = np.concatenate(
        [P_BASIS.reshape(-1), (P_BASIS * P_BASIS).sum(axis=0)]).astype(np.float32)[None, :]

    in_maps = []
    for cidx in range(N_CORES):
        sl = slice(cidx * RAYS_PER_CORE, (cidx + 1) * RAYS_PER_CORE)
        in_maps.append({
            "ray_o": np.ascontiguousarray(ray_o[sl]).astype(np.float32, copy=False),
            "ray_d": np.ascontiguousarray(ray_d[sl]).astype(np.float32, copy=False),
            "fg_z": np.ascontiguousarray(fg_z_vals[sl]).astype(np.float32, copy=False),
            "bg_z": np.ascontiguousarray(bg_z_vals[sl]).astype(np.float32, copy=False),
            "radii": np.ascontiguousarray(radii[sl]).astype(np.float32, copy=False),
            "pconst": pconst,
        })

    res = run_bass_kernel_spmd(nc, in_maps, core_ids=list(range(N_CORES)))
    outs = [res.results[i]["out"].reshape(RAYS_PER_CORE, NS, FOUT)
            for i in range(N_CORES)]
    return np.concatenate(outs, axis=0)



# revision 8
# speedup vs baseline: 1.2611x; 1.2611x over previous
"""MipNerf IPE encoding kernel for Trainium2 (Bass/Tile), 8-core SPMD.

Computes reference(ray_o, ray_d, fg_z_vals, bg_z_vals, radii) -> [2048, 64, 768]:
  fg: diagonal-cov cone cast + diagonal IPE (48 sin + 48 cos features)
  bg: full-cov cone cast + contraction Jacobian + icosahedral-basis IPE
      (336 sin + 336 cos features)

Sharding: embarrassingly data-parallel over rays; 256 rays per core.

v3 design (per 128-ray tile; 8-sample eighths stream through):
  - algebra -> yvb f32 [s*24+f] (variances; rank-4 outer-product form) and
    u0 i32 [s*24+f] (fixed-point angle fractions; bg needs no range
    reduction since |y|<2 rad after contraction -> direct f32->i32 convert)
  - us: i32 shift cascade u0<<j, j-major [j*192+s*24+f] (exact angle doubling)
  - S = Sin2pi(us * 2^-32) -> f16, one ACT per eighth (i32 path is exact)
  - cos via double-angle identity: C_j = 1 - 2*S_{j-1}^2; level-0 uses a tiny
    half-angle ACT sinh0 = Sin2pi(us_0 * 2^-33).  D = -2*S^2 via one stt;
    the +1 folds into the cos product: out_c = (D+1)*E (stt, same cost as TT)
  - E = exp(-0.5*4^j*yv) f16 j-major per half (f32-in ACTs, 4 per eighth)
  - products: 2 TT-class f16 ops/eighth writing s-major [s*768 + {sin|cos}
    *384 + j*24+f] (2x_1p mode; strided srcs, interleaved dst)
  - out DRAM is float16 with permuted columns; host unpermutes + casts f32
    (both within the 2e-2 tolerance; halves the output DMA bytes)
  - no GpSimd (SBUF port contention with DVE), no uc/int-phase ops
"""

import numpy as np

import concourse.bass as bass
import concourse.tile as tile
from concourse import mybir

F32 = mybir.dt.float32
F16 = mybir.dt.float16
I32 = mybir.dt.int32
U32 = mybir.dt.uint32
U16 = mybir.dt.uint16
AF = mybir.ActivationFunctionType
OP = mybir.AluOpType

MAGIC_RND = 12582912.0          # 1.5 * 2^23, float32 round-to-nearest trick
RSQRT_MAGIC = 0x5F3759DF
INV2PI = float(1.0 / (2.0 * np.pi))
K32 = float(2.0 ** 32 / (2.0 * np.pi))   # rad -> i32 turn fraction
TINY = 1e-6

# icosahedral basis (matches reference.py)
P_BASIS = np.array([
    0.8506508, 0.0, 0.5257311, 0.809017, 0.5, 0.309017, 0.5257311, 0.8506508, 0.0,
    1.0, 0.0, 0.0, 0.809017, 0.5, -0.309017, 0.8506508, 0.0, -0.5257311, 0.309017,
    0.809017, -0.5, 0.0, 0.5257311, -0.8506508, 0.5, 0.309017, -0.809017, 0.0, 1.0,
    0.0, -0.5257311, 0.8506508, 0.0, -0.309017, 0.809017, -0.5, 0.0, 0.5257311,
    0.8506508, -0.309017, 0.809017, 0.5, 0.309017, 0.809017, 0.5, 0.5, 0.309017,
    0.809017, 0.5, -0.309017, 0.809017, 0.0, 0.0, 1.0, -0.5, 0.309017, 0.809017,
    -0.809017, 0.5, 0.309017, -0.809017, 0.5, -0.309017], dtype=np.float32).reshape(3, 21)

N_CORES = 8
RAYS_PER_CORE = 256
NS = 64           # samples per ray
NL = 16           # frequency levels
NF = 24           # 21 bg basis dims + 3 fg axes
HALF = 32         # samples per half-tile
EI = 8            # samples per eighth (output block)
W = NF * EI       # 192: one level-block per eighth
FOUT = 768


def _out_perm():
    """inv[c]: kernel col holding reference col c (per 768-block)."""
    inv = np.zeros(FOUT, dtype=np.int64)
    for half in range(2):
        for j in range(NL):
            for f in range(NF):
                p = half * 384 + j * 24 + f
                if f < 21:
                    ref = 96 + half * 336 + j * 21 + f
                else:
                    ref = half * 48 + j * 3 + (f - 21)
                inv[ref] = p
    return inv


OUT_PERM = _out_perm()


# ---------------------------------------------------------------------------
# walrus workarounds
# ---------------------------------------------------------------------------

_PATCHED = False


def _apply_patches():
    """1) split >1 sem-waits per instruction (this walrus rejects multi-wait
    instructions);  2) rewrite sentinel Arctan activations into Sin2pi."""
    global _PATCHED
    if _PATCHED:
        return
    _PATCHED = True

    import concourse.bass2jax as bass2jax

    orig_compile = bass2jax.compile_bir_kernel

    def patched_compile(bir_json, tmpdir, neff_name="file.neff"):
        if isinstance(bir_json, bytes):
            bir_json = bir_json.replace(b'"func":"Arctan"', b'"func":"Sin2pi"')
        else:
            bir_json = bir_json.replace('"func":"Arctan"', '"func":"Sin2pi"')
        return orig_compile(bir_json, tmpdir, neff_name=neff_name)

    bass2jax.compile_bir_kernel = patched_compile


_waitsplit_ctr = [0]


def _split_sync_waits(nc, max_waits=1):
    n_split = 0
    for fn in nc.m.functions:
        for bb in fn.blocks:
            il = bb.instructions
            i = 0
            while i < len(il):
                ins = il[i]
                si = ins.sync_info
                waits = list(si.on_wait) if si is not None else []
                if len(waits) > max_waits:
                    extra, keep = waits[:-max_waits], waits[-max_waits:]
                    pos = i
                    for j in range(0, len(extra), max_waits):
                        chunk = extra[j:j + max_waits]
                        _waitsplit_ctr[0] += 1
                        nop = mybir.InstNoOp(
                            name=f"waitsplit_{_waitsplit_ctr[0]}", ins=[], outs=[])
                        nop.engine = ins.engine
                        nop.sync_info = mybir.SyncInfo(on_wait=chunk, on_update=[])
                        nc.register_instruction(nop, overwrite=True)
                        il.insert(pos, nop)
                        pos += 1
                        i += 1
                    ins.sync_info = mybir.SyncInfo(
                        on_wait=keep, on_update=list(si.on_update))
                    n_split += 1
                i += 1
    return n_split


# ---------------------------------------------------------------------------
# AP helpers
# ---------------------------------------------------------------------------

def _ap(base, offset_elems, dims):
    """Custom AP over a tile/AP: keep partition dim, replace free dims."""
    return bass.AP(tensor=base.tensor, offset=base.offset + offset_elems,
                   ap=[base.ap[0]] + [list(d) for d in dims])


# ---------------------------------------------------------------------------
# kernel body
# ---------------------------------------------------------------------------

def _moments(nc, wide, z, r2, out_tm2, out_tv, out_rv, n):
    """Frustum moments -> t_mean2 (=2*t_mean), t_var, r_var [128, n].
    z is a [128, 2*(NS+1)] concat of fg|bg z_vals; t0/t1 are 2-dim APs."""
    t0 = _ap(z[:], 0, [[NS + 1, 2], [1, NS]])
    t1 = _ap(z[:], 1, [[NS + 1, 2], [1, NS]])
    sm = wide.tile([128, n], F32, tag="mo_a")
    nc.vector.tensor_tensor(out=sm[:], in0=t0, in1=t1, op=OP.add)
    df = wide.tile([128, n], F32, tag="mo_b")
    nc.vector.tensor_tensor(out=df[:], in0=t1, in1=t0, op=OP.subtract)
    sm2 = wide.tile([128, n], F32, tag="mo_c")
    nc.vector.tensor_tensor(out=sm2[:], in0=sm[:], in1=sm[:], op=OP.mult)
    df2 = wide.tile([128, n], F32, tag="mo_d")
    nc.vector.tensor_tensor(out=df2[:], in0=df[:], in1=df[:], op=OP.mult)
    # denom4 = 3*sm2 + df2
    den4 = wide.tile([128, n], F32, tag="mo_e")
    nc.vector.scalar_tensor_tensor(out=den4[:], in0=sm2[:], scalar=3.0,
                                   in1=df2[:], op0=OP.mult, op1=OP.add)
    rden4 = wide.tile([128, n], F32, tag="mo_f")
    nc.vector.reciprocal(out=rden4[:], in_=den4[:])
    u1 = wide.tile([128, n], F32, tag="mo_g")
    nc.vector.tensor_tensor(out=u1[:], in0=df2[:], in1=rden4[:], op=OP.mult)
    # t_mean2 = sm * (1 + 2*u1)
    tmp = wide.tile([128, n], F32, tag="mo_h")
    nc.vector.tensor_scalar(out=tmp[:], in0=u1[:], scalar1=2.0, scalar2=1.0,
                            op0=OP.mult, op1=OP.add)
    nc.vector.tensor_tensor(out=out_tm2[:], in0=sm[:], in1=tmp[:], op=OP.mult)
    # t_var = df2/12 - (4/15) * u1^2 * (den4 - 1.25*df2)
    u1sq = wide.tile([128, n], F32, tag="mo_h")
    nc.vector.tensor_tensor(out=u1sq[:], in0=u1[:], in1=u1[:], op=OP.mult)
    g2 = wide.tile([128, n], F32, tag="mo_a")
    nc.vector.scalar_tensor_tensor(out=g2[:], in0=df2[:], scalar=-1.25,
                                   in1=den4[:], op0=OP.mult, op1=OP.add)
    g3 = wide.tile([128, n], F32, tag="mo_c")
    nc.vector.tensor_tensor(out=g3[:], in0=u1sq[:], in1=g2[:], op=OP.mult)
    g5 = wide.tile([128, n], F32, tag="mo_e")
    nc.vector.tensor_scalar_mul(out=g5[:], in0=df2[:], scalar1=float(1.0 / 12.0))
    nc.vector.scalar_tensor_tensor(out=out_tv[:], in0=g3[:], scalar=float(-4.0 / 15.0),
                                   in1=g5[:], op0=OP.mult, op1=OP.add)
    # r_var = r2 * (sm2/16 + (5/48)*df2 - (1/15)*u1*df2)
    h1 = wide.tile([128, n], F32, tag="mo_a")
    nc.vector.tensor_tensor(out=h1[:], in0=u1[:], in1=df2[:], op=OP.mult)
    h2 = wide.tile([128, n], F32, tag="mo_c")
    nc.vector.tensor_scalar_mul(out=h2[:], in0=sm2[:], scalar1=float(1.0 / 16.0))
    h4 = wide.tile([128, n], F32, tag="mo_e")
    nc.vector.scalar_tensor_tensor(out=h4[:], in0=df2[:], scalar=float(5.0 / 48.0),
                                   in1=h2[:], op0=OP.mult, op1=OP.add)
    h5 = wide.tile([128, n], F32, tag="mo_a")
    nc.vector.scalar_tensor_tensor(out=h5[:], in0=h1[:], scalar=float(-1.0 / 15.0),
                                   in1=h4[:], op0=OP.mult, op1=OP.add)
    nc.vector.tensor_scalar_mul(out=out_rv[:], in0=h5[:], scalar1=r2[:])


def build_kernel():
    """Build the 8-core SPMD Bass module (per-core: 256 rays)."""
    _apply_patches()
    nc = bass.Bass(dynamic_dma_scratch_size=4096)

    ray_o = nc.dram_tensor("ray_o", [RAYS_PER_CORE, 3], F32, kind="ExternalInput")
    ray_d = nc.dram_tensor("ray_d", [RAYS_PER_CORE, 3], F32, kind="ExternalInput")
    fg_z = nc.dram_tensor("fg_z", [RAYS_PER_CORE, NS + 1], F32, kind="ExternalInput")
    bg_z = nc.dram_tensor("bg_z", [RAYS_PER_CORE, NS + 1], F32, kind="ExternalInput")
    radii = nc.dram_tensor("radii", [RAYS_PER_CORE, 1], F32, kind="ExternalInput")
    pconst = nc.dram_tensor("pconst", [1, 84], F32, kind="ExternalInput")
    out = nc.dram_tensor("out", [RAYS_PER_CORE, NS * FOUT], F16, kind="ExternalOutput")

    with tile.TileContext(nc) as tc:
        import contextlib
        ctx = contextlib.ExitStack()
        with ctx:
            consts = ctx.enter_context(tc.tile_pool(name="consts", bufs=1))
            cols = ctx.enter_context(tc.tile_pool(name="cols", bufs=1))
            wide = ctx.enter_context(tc.tile_pool(name="wide", bufs=2))
            base = ctx.enter_context(tc.tile_pool(name="base", bufs=1))
            yvp = ctx.enter_context(tc.tile_pool(name="yvp", bufs=1))
            u0p = ctx.enter_context(tc.tile_pool(name="u0p", bufs=1))
            ep = ctx.enter_context(tc.tile_pool(name="ep", bufs=2))
            usp = ctx.enter_context(tc.tile_pool(name="usp", bufs=2))
            sp = ctx.enter_context(tc.tile_pool(name="sp", bufs=3))
            dp = ctx.enter_context(tc.tile_pool(name="dp", bufs=2))
            outp = ctx.enter_context(tc.tile_pool(name="outp", bufs=2))

            # constants
            pc = consts.tile([128, 84], F32)
            pca = pconst[:, :]
            nc.sync.dma_start(out=pc[:], in_=bass.AP(
                tensor=pca.tensor, offset=pca.offset, ap=[[0, 128], [1, 84]]))
            magic_u = consts.tile([128, 1], U32)
            nc.vector.memset(magic_u, RSQRT_MAGIC)

            yvbs, u0s = [], []
            for t in range(2):
                r0 = t * 128

                # ---------------- load inputs ----------------
                z2 = base.tile([128, 2 * (NS + 1)], F32, tag="z2")
                nc.sync.dma_start(out=z2[:, 0:NS + 1], in_=fg_z[r0:r0 + 128, :])
                nc.sync.dma_start(out=z2[:, NS + 1:2 * (NS + 1)],
                                  in_=bg_z[r0:r0 + 128, :])
                o3 = base.tile([128, 3], F32, tag="o3")
                nc.sync.dma_start(out=o3[:], in_=ray_o[r0:r0 + 128, :])
                d3 = base.tile([128, 3], F32, tag="d3")
                nc.sync.dma_start(out=d3[:], in_=ray_d[r0:r0 + 128, :])
                rad = base.tile([128, 1], F32, tag="rad")
                nc.sync.dma_start(out=rad[:], in_=radii[r0:r0 + 128, :])

                # ---------------- per-ray scalars ----------------
                r2 = cols.tile([128, 1], F32, tag="r2")
                nc.vector.tensor_tensor(out=r2[:], in0=rad[:], in1=rad[:], op=OP.mult)
                dk2 = cols.tile([128, 3], F32, tag="dk2")
                nc.vector.tensor_tensor(out=dk2[:], in0=d3[:], in1=d3[:], op=OP.mult)
                dmag = cols.tile([128, 1], F32, tag="dmag")
                nc.vector.tensor_tensor(out=dmag[:], in0=dk2[:, 0:1], in1=dk2[:, 1:2], op=OP.add)
                nc.vector.tensor_tensor(out=dmag[:], in0=dmag[:], in1=dk2[:, 2:3], op=OP.add)
                nc.vector.tensor_scalar_max(out=dmag[:], in0=dmag[:], scalar1=1e-8)
                rdmag = cols.tile([128, 1], F32, tag="rdmag")
                nc.vector.reciprocal(out=rdmag[:], in_=dmag[:])
                hd3 = cols.tile([128, 3], F32, tag="hd3")
                nc.vector.tensor_scalar_mul(out=hd3[:], in0=d3[:], scalar1=0.5)

                # e = d @ P  [128, 21], esq; oP = o @ P; oPe; oP2
                e21 = cols.tile([128, 21], F32, tag="e21")
                nc.vector.tensor_scalar_mul(out=e21[:], in0=pc[:, 0:21], scalar1=d3[:, 0:1])
                tmp21 = cols.tile([128, 21], F32, tag="tmp21")
                nc.vector.tensor_scalar_mul(out=tmp21[:], in0=pc[:, 21:42], scalar1=d3[:, 1:2])
                nc.vector.tensor_tensor(out=e21[:], in0=e21[:], in1=tmp21[:], op=OP.add)
                nc.vector.tensor_scalar_mul(out=tmp21[:], in0=pc[:, 42:63], scalar1=d3[:, 2:3])
                nc.vector.tensor_tensor(out=e21[:], in0=e21[:], in1=tmp21[:], op=OP.add)
                esq = cols.tile([128, 21], F32, tag="esq")
                nc.vector.tensor_tensor(out=esq[:], in0=e21[:], in1=e21[:], op=OP.mult)
                oP = cols.tile([128, 21], F32, tag="oP")
                nc.vector.tensor_scalar_mul(out=oP[:], in0=pc[:, 0:21], scalar1=o3[:, 0:1])
                nc.vector.tensor_scalar_mul(out=tmp21[:], in0=pc[:, 21:42], scalar1=o3[:, 1:2])
                nc.vector.tensor_tensor(out=oP[:], in0=oP[:], in1=tmp21[:], op=OP.add)
                nc.vector.tensor_scalar_mul(out=tmp21[:], in0=pc[:, 42:63], scalar1=o3[:, 2:3])
                nc.vector.tensor_tensor(out=oP[:], in0=oP[:], in1=tmp21[:], op=OP.add)
                oPe = cols.tile([128, 21], F32, tag="oPe")
                nc.vector.tensor_tensor(out=oPe[:], in0=oP[:], in1=e21[:], op=OP.mult)
                oP2 = cols.tile([128, 21], F32, tag="oP2")
                nc.vector.tensor_tensor(out=oP2[:], in0=oP[:], in1=oP[:], op=OP.mult)

                # ---------------- moments ----------------
                tm2a = cols.tile([128, 2 * NS], F32, tag="tm2a")
                tva = cols.tile([128, 2 * NS], F32, tag="tva")
                rva = cols.tile([128, 2 * NS], F32, tag="rva")
                _moments(nc, wide, z2, r2, tm2a, tva, rva, n=2 * NS)
                tm2f, tm2b = tm2a[:, 0:NS], tm2a[:, NS:2 * NS]
                tvf, tvb = tva[:, 0:NS], tva[:, NS:2 * NS]
                rvf, rvb = rva[:, 0:NS], rva[:, NS:2 * NS]

                yvb = yvp.tile([128, NF * NS], F32, tag=f"yv{t}")
                yvbs.append(yvb)
                u0 = u0p.tile([128, NF * NS], I32, tag=f"u0{t}")
                u0s.append(u0)

                # ---------------- fg: cov_diag + angles ----------------
                alf = wide.tile([128, NS], F32, tag="mo_b")
                nc.vector.tensor_scalar_mul(out=alf[:], in0=rvf, scalar1=rdmag[:])
                nc.vector.tensor_tensor(out=alf[:], in0=tvf, in1=alf[:], op=OP.subtract)
                ybf = cols.tile([128, 3 * NS], F32, tag="ybf")   # [s*3+k]
                for k in range(3):
                    # m_k = tm2f * halfd_k + o_k  (compact [s*3+k])
                    nc.vector.tensor_scalar(
                        out=_ap(ybf[:], k, [[3, NS]]), in0=tm2f,
                        scalar1=hd3[:, k:k + 1], scalar2=o3[:, k:k + 1],
                        op0=OP.mult, op1=OP.add)
                    # cd_k = alf * dk2_k + rvf  -> yvb fg col 21+k
                    nc.vector.scalar_tensor_tensor(
                        out=_ap(yvb[:], 21 + k, [[NF, NS]]), in0=alf[:],
                        scalar=dk2[:, k:k + 1], in1=rvf, op0=OP.mult, op1=OP.add)
                # fg angle round chain: f0 = t - round(t), t = yb*inv2pi
                ttf = cols.tile([128, 3 * NS], F32, tag="ttf")
                nc.vector.tensor_scalar(out=ttf[:], in0=ybf[:], scalar1=INV2PI,
                                        scalar2=MAGIC_RND, op0=OP.mult, op1=OP.add)
                nc.vector.tensor_scalar(out=ttf[:], in0=ttf[:], scalar1=MAGIC_RND,
                                        scalar2=None, op0=OP.subtract)
                nc.vector.scalar_tensor_tensor(out=ybf[:], in0=ybf[:], scalar=INV2PI,
                                               in1=ttf[:], op0=OP.mult, op1=OP.subtract)
                # u0 fg cols (strided dst [s*24 + 21+k])
                nc.vector.tensor_scalar_mul(
                    out=_ap(u0[:], 21, [[NF, NS], [1, 3]]), in0=ybf[:],
                    scalar1=float(2.0 ** 32))

                # ---------------- bg: contraction scalars ----------------
                X = base.tile([128, 3 * NS], F32, tag="mk")          # [k*64+s]
                for k in range(3):
                    nc.vector.tensor_scalar(
                        out=X[:, k * NS:(k + 1) * NS], in0=tm2b,
                        scalar1=hd3[:, k:k + 1], scalar2=o3[:, k:k + 1],
                        op0=OP.mult, op1=OP.add)
                s2 = cols.tile([128, NS], F32, tag="s2")
                nc.vector.tensor_tensor(out=s2[:], in0=X[:, 0:NS], in1=X[:, 0:NS], op=OP.mult)
                w0 = wide.tile([128, NS], F32, tag="mo_a")
                nc.vector.tensor_tensor(out=w0[:], in0=X[:, NS:2 * NS], in1=X[:, NS:2 * NS], op=OP.mult)
                nc.vector.tensor_tensor(out=s2[:], in0=s2[:], in1=w0[:], op=OP.add)
                nc.vector.tensor_tensor(out=w0[:], in0=X[:, 2 * NS:3 * NS], in1=X[:, 2 * NS:3 * NS], op=OP.mult)
                nc.vector.tensor_tensor(out=s2[:], in0=s2[:], in1=w0[:], op=OP.add)
                # h = d . X
                h = cols.tile([128, NS], F32, tag="h")
                nc.vector.tensor_scalar_mul(out=h[:], in0=X[:, 0:NS], scalar1=d3[:, 0:1])
                nc.vector.scalar_tensor_tensor(out=h[:], in0=X[:, NS:2 * NS],
                                               scalar=d3[:, 1:2], in1=h[:],
                                               op0=OP.mult, op1=OP.add)
                nc.vector.scalar_tensor_tensor(out=h[:], in0=X[:, 2 * NS:3 * NS],
                                               scalar=d3[:, 2:3], in1=h[:],
                                               op0=OP.mult, op1=OP.add)

                # rsqrt(s2): magic seed + 3 Newton iterations
                rn0 = cols.tile([128, NS], F32, tag="rn0")
                seed_u = wide.tile([128, NS], U32, tag="mo_a")
                nc.vector.tensor_scalar(out=seed_u[:], in0=s2[:].bitcast(U32),
                                        scalar1=1, scalar2=None,
                                        op0=OP.logical_shift_right)
                nc.vector.tensor_tensor(
                    out=rn0[:].bitcast(U32),
                    in0=_ap(magic_u[:], 0, [[0, NS]]),
                    in1=seed_u[:], op=OP.subtract)
                for _ in range(3):
                    nr = wide.tile([128, NS], F32, tag="mo_b")
                    nc.vector.tensor_tensor(out=nr[:], in0=s2[:], in1=rn0[:], op=OP.mult)
                    nc.vector.tensor_tensor(out=nr[:], in0=nr[:], in1=rn0[:], op=OP.mult)
                    nc.vector.tensor_scalar(out=nr[:], in0=nr[:], scalar1=-0.5,
                                            scalar2=1.5, op0=OP.mult, op1=OP.add)
                    nc.vector.tensor_tensor(out=rn0[:], in0=rn0[:], in1=nr[:], op=OP.mult)

                n0 = cols.tile([128, NS], F32, tag="n0")
                nc.vector.tensor_tensor(out=n0[:], in0=s2[:], in1=rn0[:], op=OP.mult)
                rn = cols.tile([128, NS], F32, tag="rn")
                nc.vector.tensor_scalar(out=rn[:], in0=rn0[:], scalar1=-TINY,
                                        scalar2=1.0, op0=OP.mult, op1=OP.add)
                nc.vector.tensor_tensor(out=rn[:], in0=rn0[:], in1=rn[:], op=OP.mult)
                a_ = cols.tile([128, NS], F32, tag="a")
                nc.vector.tensor_scalar(out=a_[:], in0=rn[:], scalar1=-1.0,
                                        scalar2=2.0, op0=OP.mult, op1=OP.add)
                nc.vector.tensor_tensor(out=a_[:], in0=rn[:], in1=a_[:], op=OP.mult)
                b_ = cols.tile([128, NS], F32, tag="b")
                nc.vector.tensor_scalar_add(out=b_[:], in0=rn[:], scalar1=-1.0)
                t2_ = wide.tile([128, NS], F32, tag="mo_a")
                nc.vector.tensor_tensor(out=t2_[:], in0=rn[:], in1=rn0[:], op=OP.mult)
                nc.vector.tensor_tensor(out=t2_[:], in0=t2_[:], in1=rn[:], op=OP.mult)
                nc.vector.tensor_tensor(out=b_[:], in0=t2_[:], in1=b_[:], op=OP.mult)
                nc.vector.tensor_scalar_mul(out=b_[:], in0=b_[:], scalar1=2.0)

                # alpha_b, A coefficients
                alb = cols.tile([128, NS], F32, tag="alb")
                nc.vector.tensor_scalar_mul(out=alb[:], in0=rvb, scalar1=rdmag[:])
                nc.vector.tensor_tensor(out=alb[:], in0=tvb, in1=alb[:], op=OP.subtract)
                bh = cols.tile([128, NS], F32, tag="bh")
                nc.vector.tensor_tensor(out=bh[:], in0=b_[:], in1=h[:], op=OP.mult)
                asq = wide.tile([128, NS], F32, tag="mo_a")
                nc.vector.tensor_tensor(out=asq[:], in0=a_[:], in1=a_[:], op=OP.mult)
                A1 = cols.tile([128, NS], F32, tag="A1")
                nc.vector.tensor_tensor(out=A1[:], in0=alb[:], in1=asq[:], op=OP.mult)
                A4 = cols.tile([128, NS], F32, tag="A4")
                nc.vector.tensor_tensor(out=A4[:], in0=rvb, in1=asq[:], op=OP.mult)
                A2 = cols.tile([128, NS], F32, tag="A2")
                nc.vector.tensor_tensor(out=A2[:], in0=alb[:], in1=a_[:], op=OP.mult)
                nc.vector.tensor_tensor(out=A2[:], in0=A2[:], in1=bh[:], op=OP.mult)
                nc.vector.tensor_scalar_mul(out=A2[:], in0=A2[:], scalar1=2.0)
                # A3 = alb*bh^2 + rvb*(2ab + (b*n0)^2)
                A3 = cols.tile([128, NS], F32, tag="A3")
                bn = wide.tile([128, NS], F32, tag="mo_b")
                nc.vector.tensor_tensor(out=bn[:], in0=b_[:], in1=n0[:], op=OP.mult)
                nc.vector.tensor_tensor(out=bn[:], in0=bn[:], in1=bn[:], op=OP.mult)
                ab = wide.tile([128, NS], F32, tag="mo_c")
                nc.vector.tensor_tensor(out=ab[:], in0=a_[:], in1=b_[:], op=OP.mult)
                nc.vector.scalar_tensor_tensor(out=bn[:], in0=ab[:], scalar=2.0,
                                               in1=bn[:], op0=OP.mult, op1=OP.add)
                nc.vector.tensor_tensor(out=A3[:], in0=rvb, in1=bn[:], op=OP.mult)
                bh2 = wide.tile([128, NS], F32, tag="mo_a")
                nc.vector.tensor_tensor(out=bh2[:], in0=bh[:], in1=bh[:], op=OP.mult)
                nc.vector.tensor_tensor(out=bh2[:], in0=alb[:], in1=bh2[:], op=OP.mult)
                nc.vector.tensor_tensor(out=A3[:], in0=A3[:], in1=bh2[:], op=OP.add)

                # ---- rank-4 coefficients: yv = C1*esq + C2*oPe + A3*oP2 + A4*w
                # tm = t_mean;  C1 = A1 + tm*(A2 + tm*A3);  C2 = A2 + 2*tm*A3
                tm = cols.tile([128, NS], F32, tag="tm")
                nc.vector.tensor_scalar_mul(out=tm[:], in0=tm2b, scalar1=0.5)
                v_ = wide.tile([128, NS], F32, tag="mo_b")
                nc.vector.tensor_tensor(out=v_[:], in0=tm[:], in1=A3[:], op=OP.mult)
                C2 = cols.tile([128, NS], F32, tag="C2")
                nc.vector.scalar_tensor_tensor(out=C2[:], in0=v_[:], scalar=2.0,
                                               in1=A2[:], op0=OP.mult, op1=OP.add)
                C1 = cols.tile([128, NS], F32, tag="C1")
                nc.vector.tensor_tensor(out=C1[:], in0=v_[:], in1=A2[:], op=OP.add)
                nc.vector.tensor_tensor(out=C1[:], in0=tm[:], in1=C1[:], op=OP.mult)
                nc.vector.tensor_tensor(out=C1[:], in0=C1[:], in1=A1[:], op=OP.add)
                # B1 = a/2pi, B2 = a*tm/2pi  (angle-in-turns coefficients)
                B1 = cols.tile([128, NS], F32, tag="B1")
                nc.vector.tensor_scalar_mul(out=B1[:], in0=a_[:], scalar1=INV2PI)
                B2 = cols.tile([128, NS], F32, tag="B2")
                nc.vector.tensor_tensor(out=B2[:], in0=B1[:], in1=tm[:], op=OP.mult)

                # ---- bg yv: 7 broadcast TTs into yvb strided [s*24+f], f<21
                yvb_bg = _ap(yvb[:], 0, [[NF, NS], [1, 21]])
                w1 = base.tile([128, 21 * NS], F32, tag="w1")
                C1b = _ap(C1[:], 0, [[1, NS], [0, 21]])
                C2b = _ap(C2[:], 0, [[1, NS], [0, 21]])
                A3b = _ap(A3[:], 0, [[1, NS], [0, 21]])
                A4b = _ap(A4[:], 0, [[1, NS], [0, 21]])
                esq_b = _ap(esq[:], 0, [[0, NS], [1, 21]])
                oPe_b = _ap(oPe[:], 0, [[0, NS], [1, 21]])
                oP2_b = _ap(oP2[:], 0, [[0, NS], [1, 21]])
                w_b = _ap(pc[:], 63, [[0, NS], [1, 21]])
                nc.vector.tensor_tensor(out=yvb_bg, in0=C1b, in1=esq_b, op=OP.mult)
                nc.vector.tensor_tensor(out=w1[:], in0=C2b, in1=oPe_b, op=OP.mult)
                nc.vector.tensor_tensor(out=yvb_bg, in0=yvb_bg, in1=w1[:], op=OP.add)
                nc.vector.tensor_tensor(out=w1[:], in0=A3b, in1=oP2_b, op=OP.mult)
                nc.vector.tensor_tensor(out=yvb_bg, in0=yvb_bg, in1=w1[:], op=OP.add)
                nc.vector.tensor_tensor(out=w1[:], in0=A4b, in1=w_b, op=OP.mult)
                nc.vector.tensor_tensor(out=yvb_bg, in0=yvb_bg, in1=w1[:], op=OP.add)

                # ---- bg angles pre-scaled: y' = B1*oP + B2*e  ([s*21+q])
                B1b = _ap(B1[:], 0, [[1, NS], [0, 21]])
                B2b = _ap(B2[:], 0, [[1, NS], [0, 21]])
                oP_b = _ap(oP[:], 0, [[0, NS], [1, 21]])
                e_b = _ap(e21[:], 0, [[0, NS], [1, 21]])
                y1 = base.tile([128, 21 * NS], F32, tag="y1")
                nc.vector.tensor_tensor(out=y1[:], in0=B2b, in1=e_b, op=OP.mult)
                nc.vector.tensor_tensor(out=w1[:], in0=B1b, in1=oP_b, op=OP.mult)
                nc.vector.tensor_tensor(out=y1[:], in0=y1[:], in1=w1[:], op=OP.add)
                # range-reduce (|y1| can exceed 0.5 turns when the contraction
                # blows up near the origin): f0 = y1 - round(y1)
                nc.vector.tensor_scalar(out=w1[:], in0=y1[:], scalar1=MAGIC_RND,
                                        scalar2=None, op0=OP.add)
                nc.vector.tensor_scalar(out=w1[:], in0=w1[:], scalar1=MAGIC_RND,
                                        scalar2=None, op0=OP.subtract)
                nc.vector.tensor_tensor(out=y1[:], in0=y1[:], in1=w1[:],
                                        op=OP.subtract)
                # u0 bg cols: f32 -> i32 convert of the turn fraction
                nc.vector.tensor_scalar_mul(
                    out=_ap(u0[:], 0, [[NF, NS], [1, 21]]), in0=y1[:],
                    scalar1=float(2.0 ** 32))

            # ---------------- streaming: per-eighth pipeline ----------
            # E, S, D are stored s-major: [s*384 + j*24 + f] (strided ACT
            # dst is free; makes every product operand <=3D and contiguous
            # per sample, which walrus' stt verifier requires).
            def emit_exp(E, t, hh, j0, j1):
                for j in range(j0, j1):
                    nc.scalar.activation(
                        out=_ap(E[:], j * NF, [[NL * NF, HALF], [1, NF]]),
                        in_=yvbs[t][:, hh * NF * HALF:(hh + 1) * NF * HALF],
                        func=AF.Exp, scale=float(-0.5 * (4.0 ** j)))

            def cascade(t, e_idx):
                """i32 shift cascade for one eighth -> us [j*W + s*24+f]."""
                u0 = u0s[t]
                us = usp.tile([128, NL * W], I32, tag="us")
                nc.vector.tensor_copy(out=us[:, 0:W],
                                      in_=u0[:, e_idx * W:(e_idx + 1) * W])
                for b, sh in ((1, 1), (2, 2), (4, 4), (8, 8)):
                    nc.vector.tensor_scalar(
                        out=us[:, b * W:2 * b * W], in0=us[:, 0:b * W],
                        scalar1=sh, scalar2=None,
                        op0=OP.logical_shift_left)
                return us

            ang = {}
            ecur = ep.tile([128, NL * NF * HALF], F16, tag="E")
            emit_exp(ecur, 0, 0, 0, NL)     # prologue: E for (t=0, h=0)
            for t in range(2):
                ang[t] = cascade(t, 0)
                for hh in range(2):
                    nt, nh = (t, 1) if hh == 0 else (t + 1, 0)
                    enext = None
                    if nt < 2:
                        enext = ep.tile([128, NL * NF * HALF], F16, tag="E")

                    for ee in range(4):
                        e_idx = hh * 4 + ee          # eighth within tile
                        r0 = t * 128
                        E = ecur
                        us = ang.pop(t)

                        # --- ScE: sinh0 (level-0 half angle) + S (all levels)
                        # S written s-major [s*384 + j*24 + f]
                        sh0 = sp.tile([128, W], F16, tag="sh0")
                        nc.scalar.activation(out=sh0[:], in_=us[:, 0:W],
                                             func=AF.Arctan,
                                             scale=float(2.0 ** -33))
                        S = sp.tile([128, NL * W], F16, tag="S")
                        nc.scalar.activation(
                            out=_ap(S[:], 0, [[NF, NL], [NL * NF, EI], [1, NF]]),
                            in_=us[:], func=AF.Arctan,
                            scale=float(2.0 ** -32))

                        # --- software pipeline: next eighth's cascade (DVE)
                        if e_idx < 7:
                            ang[t] = cascade(t, e_idx + 1)

                        # --- spread next half's exp instrs (4 per eighth) ---
                        if enext is not None:
                            emit_exp(enext, nt, nh, 4 * ee, 4 * ee + 4)

                        # --- D = -2*sin(theta/2)^2  (cos_j = D_j + 1) ---
                        # s-major: D[s, j] from S[s, j-1]; D[s, 0] from sh0
                        D = dp.tile([128, NL * W], F16, tag="D")
                        nc.vector.scalar_tensor_tensor(
                            out=_ap(D[:], 0, [[NL * NF, EI], [1, NF]]),
                            in0=sh0[:], scalar=-2.0,
                            in1=sh0[:], op0=OP.mult, op1=OP.mult)
                        s_lo = _ap(S[:], 0, [[NL * NF, EI], [1, (NL - 1) * NF]])
                        nc.vector.scalar_tensor_tensor(
                            out=_ap(D[:], NF, [[NL * NF, EI], [1, (NL - 1) * NF]]),
                            in0=s_lo, scalar=-2.0,
                            in1=s_lo, op0=OP.mult, op1=OP.mult)

                        # --- products into interleaved s-major out tile ---
                        ob = outp.tile([128, EI * FOUT], F16, tag="ob")
                        e_off = ee * EI * NL * NF   # E offset for this eighth
                        s_src = _ap(S[:], 0, [[NL * NF, EI], [1, 384]])
                        d_src = _ap(D[:], 0, [[NL * NF, EI], [1, 384]])
                        e_src = _ap(E[:], e_off, [[NL * NF, EI], [1, 384]])
                        o_sin = _ap(ob[:], 0, [[FOUT, EI], [1, 384]])
                        o_cos = _ap(ob[:], 384, [[FOUT, EI], [1, 384]])
                        nc.vector.tensor_tensor(out=o_sin, in0=s_src,
                                                in1=e_src, op=OP.mult)
                        nc.vector.scalar_tensor_tensor(
                            out=o_cos, in0=d_src, scalar=1.0, in1=e_src,
                            op0=OP.add, op1=OP.mult)

                        # --- DMA out (single contiguous block) ---
                        oa = out[:, :]
                        nc.sync.dma_start(
                            out=bass.AP(
                                tensor=oa.tensor,
                                offset=oa.offset + r0 * NS * FOUT + e_idx * EI * FOUT,
                                ap=[[NS * FOUT, 128], [1, EI * FOUT]]),
                            in_=ob[:])
                    ecur = enext

    _split_sync_waits(nc)
    return nc


# ---------------------------------------------------------------------------
# entry point
# ---------------------------------------------------------------------------

_NC_CACHE = []


def kernel(ray_o, ray_d, fg_z_vals, bg_z_vals, radii):
    from concourse.bass_utils import run_bass_kernel_spmd

    if not _NC_CACHE:
        _NC_CACHE.append(build_kernel())
    nc = _NC_CACHE[0]

    pconst = np.concatenate(
        [P_BASIS.reshape(-1), (P_BASIS * P_BASIS).sum(axis=0)]).astype(np.float32)[None, :]

    in_maps = []
    for cidx in range(N_CORES):
        sl = slice(cidx * RAYS_PER_CORE, (cidx + 1) * RAYS_PER_CORE)
        in_maps.append({
            "ray_o": np.ascontiguousarray(ray_o[sl]).astype(np.float32, copy=False),
            "ray_d": np.ascontiguousarray(ray_d[sl]).astype(np.float32, copy=False),
            "fg_z": np.ascontiguousarray(fg_z_vals[sl]).astype(np.float32, copy=False),
            "bg_z": np.ascontiguousarray(bg_z_vals[sl]).astype(np.float32, copy=False),
            "radii": np.ascontiguousarray(radii[sl]).astype(np.float32, copy=False),
            "pconst": pconst,
        })

    res = run_bass_kernel_spmd(nc, in_maps, core_ids=list(range(N_CORES)))
    outs = [res.results[i]["out"].reshape(RAYS_PER_CORE, NS, FOUT)
            for i in range(N_CORES)]
    raw = np.concatenate(outs, axis=0)
    return raw[..., OUT_PERM].astype(np.float32)


# revision 9
# speedup vs baseline: 1.4977x; 1.1877x over previous
"""MipNerf IPE encoding kernel for Trainium2 (Bass/Tile), 8-core SPMD.

Computes reference(ray_o, ray_d, fg_z_vals, bg_z_vals, radii) -> [2048, 64, 768]:
  fg: diagonal-cov cone cast + diagonal IPE (48 sin + 48 cos features)
  bg: full-cov cone cast + contraction Jacobian + icosahedral-basis IPE
      (336 sin + 336 cos features)

Sharding: embarrassingly data-parallel over rays; 256 rays per core.

v3 design (per 128-ray tile; 8-sample eighths stream through):
  - algebra -> yvb f32 [s*24+f] (variances; rank-4 outer-product form) and
    u0 i32 [s*24+f] (fixed-point angle fractions; bg needs no range
    reduction since |y|<2 rad after contraction -> direct f32->i32 convert)
  - us: i32 shift cascade u0<<j, j-major [j*192+s*24+f] (exact angle doubling)
  - S = Sin2pi(us * 2^-32) -> f16, one ACT per eighth (i32 path is exact)
  - cos via double-angle identity: C_j = 1 - 2*S_{j-1}^2; level-0 uses a tiny
    half-angle ACT sinh0 = Sin2pi(us_0 * 2^-33).  D = -2*S^2 via one stt;
    the +1 folds into the cos product: out_c = (D+1)*E (stt, same cost as TT)
  - E = exp(-0.5*4^j*yv) f16 j-major per half (f32-in ACTs, 4 per eighth)
  - products: 2 TT-class f16 ops/eighth writing s-major [s*768 + {sin|cos}
    *384 + j*24+f] (2x_1p mode; strided srcs, interleaved dst)
  - out DRAM is float16 with permuted columns; host unpermutes + casts f32
    (both within the 2e-2 tolerance; halves the output DMA bytes)
  - no GpSimd (SBUF port contention with DVE), no uc/int-phase ops
"""

import numpy as np

import concourse.bass as bass
import concourse.tile as tile
from concourse import mybir

F32 = mybir.dt.float32
F16 = mybir.dt.float16
I32 = mybir.dt.int32
U32 = mybir.dt.uint32
U16 = mybir.dt.uint16
AF = mybir.ActivationFunctionType
OP = mybir.AluOpType

MAGIC_RND = 12582912.0          # 1.5 * 2^23, float32 round-to-nearest trick
RSQRT_MAGIC = 0x5F3759DF
INV2PI = float(1.0 / (2.0 * np.pi))
K32 = float(2.0 ** 32 / (2.0 * np.pi))   # rad -> i32 turn fraction
TINY = 1e-6

# icosahedral basis (matches reference.py)
P_BASIS = np.array([
    0.8506508, 0.0, 0.5257311, 0.809017, 0.5, 0.309017, 0.5257311, 0.8506508, 0.0,
    1.0, 0.0, 0.0, 0.809017, 0.5, -0.309017, 0.8506508, 0.0, -0.5257311, 0.309017,
    0.809017, -0.5, 0.0, 0.5257311, -0.8506508, 0.5, 0.309017, -0.809017, 0.0, 1.0,
    0.0, -0.5257311, 0.8506508, 0.0, -0.309017, 0.809017, -0.5, 0.0, 0.5257311,
    0.8506508, -0.309017, 0.809017, 0.5, 0.309017, 0.809017, 0.5, 0.5, 0.309017,
    0.809017, 0.5, -0.309017, 0.809017, 0.0, 0.0, 1.0, -0.5, 0.309017, 0.809017,
    -0.809017, 0.5, 0.309017, -0.809017, 0.5, -0.309017], dtype=np.float32).reshape(3, 21)

N_CORES = 8
RAYS_PER_CORE = 256
NS = 64           # samples per ray
NL = 16           # frequency levels
NF = 24           # 21 bg basis dims + 3 fg axes
HALF = 32         # samples per half-tile
EI = 8            # samples per eighth (output block)
W = NF * EI       # 192: one level-block per eighth
FOUT = 768


def _out_perm():
    """inv[c]: kernel col holding reference col c (per 768-block)."""
    inv = np.zeros(FOUT, dtype=np.int64)
    for half in range(2):
        for j in range(NL):
            for f in range(NF):
                p = half * 384 + j * 24 + f
                if f < 21:
                    ref = 96 + half * 336 + j * 21 + f
                else:
                    ref = half * 48 + j * 3 + (f - 21)
                inv[ref] = p
    return inv


OUT_PERM = _out_perm()


# ---------------------------------------------------------------------------
# walrus workarounds
# ---------------------------------------------------------------------------

_PATCHED = False


def _apply_patches():
    """1) split >1 sem-waits per instruction (this walrus rejects multi-wait
    instructions);  2) rewrite sentinel Arctan activations into Sin2pi."""
    global _PATCHED
    if _PATCHED:
        return
    _PATCHED = True

    import concourse.bass2jax as bass2jax

    orig_compile = bass2jax.compile_bir_kernel

    def patched_compile(bir_json, tmpdir, neff_name="file.neff"):
        if isinstance(bir_json, bytes):
            bir_json = bir_json.replace(b'"func":"Arctan"', b'"func":"Sin2pi"')
        else:
            bir_json = bir_json.replace('"func":"Arctan"', '"func":"Sin2pi"')
        return orig_compile(bir_json, tmpdir, neff_name=neff_name)

    bass2jax.compile_bir_kernel = patched_compile


_waitsplit_ctr = [0]


def _split_sync_waits(nc, max_waits=1):
    n_split = 0
    for fn in nc.m.functions:
        for bb in fn.blocks:
            il = bb.instructions
            i = 0
            while i < len(il):
                ins = il[i]
                si = ins.sync_info
                waits = list(si.on_wait) if si is not None else []
                if len(waits) > max_waits:
                    extra, keep = waits[:-max_waits], waits[-max_waits:]
                    pos = i
                    for j in range(0, len(extra), max_waits):
                        chunk = extra[j:j + max_waits]
                        _waitsplit_ctr[0] += 1
                        nop = mybir.InstNoOp(
                            name=f"waitsplit_{_waitsplit_ctr[0]}", ins=[], outs=[])
                        nop.engine = ins.engine
                        nop.sync_info = mybir.SyncInfo(on_wait=chunk, on_update=[])
                        nc.register_instruction(nop, overwrite=True)
                        il.insert(pos, nop)
                        pos += 1
                        i += 1
                    ins.sync_info = mybir.SyncInfo(
                        on_wait=keep, on_update=list(si.on_update))
                    n_split += 1
                i += 1
    return n_split


# ---------------------------------------------------------------------------
# AP helpers
# ---------------------------------------------------------------------------

def _ap(base, offset_elems, dims):
    """Custom AP over a tile/AP: keep partition dim, replace free dims."""
    return bass.AP(tensor=base.tensor, offset=base.offset + offset_elems,
                   ap=[base.ap[0]] + [list(d) for d in dims])


# ---------------------------------------------------------------------------
# kernel body
# ---------------------------------------------------------------------------

def _moments(nc, wide, z, r2, out_tm2, out_tv, out_rv, n):
    """Frustum moments -> t_mean2 (=2*t_mean), t_var, r_var [128, n].
    z is a [128, 2*(NS+1)] concat of fg|bg z_vals; t0/t1 are 2-dim APs."""
    t0 = _ap(z[:], 0, [[NS + 1, 2], [1, NS]])
    t1 = _ap(z[:], 1, [[NS + 1, 2], [1, NS]])
    sm = wide.tile([128, n], F32, tag="mo_a")
    nc.vector.tensor_tensor(out=sm[:], in0=t0, in1=t1, op=OP.add)
    df = wide.tile([128, n], F32, tag="mo_b")
    nc.vector.tensor_tensor(out=df[:], in0=t1, in1=t0, op=OP.subtract)
    sm2 = wide.tile([128, n], F32, tag="mo_c")
    nc.vector.tensor_tensor(out=sm2[:], in0=sm[:], in1=sm[:], op=OP.mult)
    df2 = wide.tile([128, n], F32, tag="mo_d")
    nc.vector.tensor_tensor(out=df2[:], in0=df[:], in1=df[:], op=OP.mult)
    # denom4 = 3*sm2 + df2
    den4 = wide.tile([128, n], F32, tag="mo_e")
    nc.vector.scalar_tensor_tensor(out=den4[:], in0=sm2[:], scalar=3.0,
                                   in1=df2[:], op0=OP.mult, op1=OP.add)
    rden4 = wide.tile([128, n], F32, tag="mo_f")
    nc.vector.reciprocal(out=rden4[:], in_=den4[:])
    u1 = wide.tile([128, n], F32, tag="mo_g")
    nc.vector.tensor_tensor(out=u1[:], in0=df2[:], in1=rden4[:], op=OP.mult)
    # t_mean2 = sm * (1 + 2*u1)
    tmp = wide.tile([128, n], F32, tag="mo_h")
    nc.vector.tensor_scalar(out=tmp[:], in0=u1[:], scalar1=2.0, scalar2=1.0,
                            op0=OP.mult, op1=OP.add)
    nc.vector.tensor_tensor(out=out_tm2[:], in0=sm[:], in1=tmp[:], op=OP.mult)
    # t_var = df2/12 - (4/15) * u1^2 * (den4 - 1.25*df2)
    u1sq = wide.tile([128, n], F32, tag="mo_h")
    nc.vector.tensor_tensor(out=u1sq[:], in0=u1[:], in1=u1[:], op=OP.mult)
    g2 = wide.tile([128, n], F32, tag="mo_a")
    nc.vector.scalar_tensor_tensor(out=g2[:], in0=df2[:], scalar=-1.25,
                                   in1=den4[:], op0=OP.mult, op1=OP.add)
    g3 = wide.tile([128, n], F32, tag="mo_c")
    nc.vector.tensor_tensor(out=g3[:], in0=u1sq[:], in1=g2[:], op=OP.mult)
    g5 = wide.tile([128, n], F32, tag="mo_e")
    nc.vector.tensor_scalar_mul(out=g5[:], in0=df2[:], scalar1=float(1.0 / 12.0))
    nc.vector.scalar_tensor_tensor(out=out_tv[:], in0=g3[:], scalar=float(-4.0 / 15.0),
                                   in1=g5[:], op0=OP.mult, op1=OP.add)
    # r_var = r2 * (sm2/16 + (5/48)*df2 - (1/15)*u1*df2)
    h1 = wide.tile([128, n], F32, tag="mo_a")
    nc.vector.tensor_tensor(out=h1[:], in0=u1[:], in1=df2[:], op=OP.mult)
    h2 = wide.tile([128, n], F32, tag="mo_c")
    nc.vector.tensor_scalar_mul(out=h2[:], in0=sm2[:], scalar1=float(1.0 / 16.0))
    h4 = wide.tile([128, n], F32, tag="mo_e")
    nc.vector.scalar_tensor_tensor(out=h4[:], in0=df2[:], scalar=float(5.0 / 48.0),
                                   in1=h2[:], op0=OP.mult, op1=OP.add)
    h5 = wide.tile([128, n], F32, tag="mo_a")
    nc.vector.scalar_tensor_tensor(out=h5[:], in0=h1[:], scalar=float(-1.0 / 15.0),
                                   in1=h4[:], op0=OP.mult, op1=OP.add)
    nc.vector.tensor_scalar_mul(out=out_rv[:], in0=h5[:], scalar1=r2[:])


def build_kernel():
    """Build the 8-core SPMD Bass module (per-core: 256 rays)."""
    _apply_patches()
    nc = bass.Bass(dynamic_dma_scratch_size=4096)

    ray_o = nc.dram_tensor("ray_o", [RAYS_PER_CORE, 3], F32, kind="ExternalInput")
    ray_d = nc.dram_tensor("ray_d", [RAYS_PER_CORE, 3], F32, kind="ExternalInput")
    fg_z = nc.dram_tensor("fg_z", [RAYS_PER_CORE, NS + 1], F32, kind="ExternalInput")
    bg_z = nc.dram_tensor("bg_z", [RAYS_PER_CORE, NS + 1], F32, kind="ExternalInput")
    radii = nc.dram_tensor("radii", [RAYS_PER_CORE, 1], F32, kind="ExternalInput")
    pconst = nc.dram_tensor("pconst", [1, 84], F32, kind="ExternalInput")
    out = nc.dram_tensor("out", [RAYS_PER_CORE, NS * FOUT], F16, kind="ExternalOutput")

    with tile.TileContext(nc) as tc:
        import contextlib
        ctx = contextlib.ExitStack()
        with ctx:
            consts = ctx.enter_context(tc.tile_pool(name="consts", bufs=1))
            cols = ctx.enter_context(tc.tile_pool(name="cols", bufs=1))
            wide = ctx.enter_context(tc.tile_pool(name="wide", bufs=2))
            base = ctx.enter_context(tc.tile_pool(name="base", bufs=1))
            yvp = ctx.enter_context(tc.tile_pool(name="yvp", bufs=1))
            u0p = ctx.enter_context(tc.tile_pool(name="u0p", bufs=1))
            ep = ctx.enter_context(tc.tile_pool(name="ep", bufs=2))
            usp = ctx.enter_context(tc.tile_pool(name="usp", bufs=2))
            sp = ctx.enter_context(tc.tile_pool(name="sp", bufs=3))
            dp = ctx.enter_context(tc.tile_pool(name="dp", bufs=2))
            outp = ctx.enter_context(tc.tile_pool(name="outp", bufs=2))

            # constants
            pc = consts.tile([128, 84], F32)
            pca = pconst[:, :]
            nc.sync.dma_start(out=pc[:], in_=bass.AP(
                tensor=pca.tensor, offset=pca.offset, ap=[[0, 128], [1, 84]]))
            magic_u = consts.tile([128, 1], U32)
            nc.vector.memset(magic_u, RSQRT_MAGIC)

            yvbs, u0s = [], []
            for t in range(2):
                r0 = t * 128

                # ---------------- load inputs ----------------
                z2 = base.tile([128, 2 * (NS + 1)], F32, tag="z2")
                nc.sync.dma_start(out=z2[:, 0:NS + 1], in_=fg_z[r0:r0 + 128, :])
                nc.sync.dma_start(out=z2[:, NS + 1:2 * (NS + 1)],
                                  in_=bg_z[r0:r0 + 128, :])
                o3 = base.tile([128, 3], F32, tag="o3")
                nc.sync.dma_start(out=o3[:], in_=ray_o[r0:r0 + 128, :])
                d3 = base.tile([128, 3], F32, tag="d3")
                nc.sync.dma_start(out=d3[:], in_=ray_d[r0:r0 + 128, :])
                rad = base.tile([128, 1], F32, tag="rad")
                nc.sync.dma_start(out=rad[:], in_=radii[r0:r0 + 128, :])

                # ---------------- per-ray scalars ----------------
                r2 = cols.tile([128, 1], F32, tag="r2")
                nc.vector.tensor_tensor(out=r2[:], in0=rad[:], in1=rad[:], op=OP.mult)
                dk2 = cols.tile([128, 3], F32, tag="dk2")
                nc.vector.tensor_tensor(out=dk2[:], in0=d3[:], in1=d3[:], op=OP.mult)
                dmag = cols.tile([128, 1], F32, tag="dmag")
                nc.vector.tensor_tensor(out=dmag[:], in0=dk2[:, 0:1], in1=dk2[:, 1:2], op=OP.add)
                nc.vector.tensor_tensor(out=dmag[:], in0=dmag[:], in1=dk2[:, 2:3], op=OP.add)
                nc.vector.tensor_scalar_max(out=dmag[:], in0=dmag[:], scalar1=1e-8)
                rdmag = cols.tile([128, 1], F32, tag="rdmag")
                nc.vector.reciprocal(out=rdmag[:], in_=dmag[:])
                hd3 = cols.tile([128, 3], F32, tag="hd3")
                nc.vector.tensor_scalar_mul(out=hd3[:], in0=d3[:], scalar1=0.5)

                # e = d @ P  [128, 21], esq; oP = o @ P; oPe; oP2
                e21 = cols.tile([128, 21], F32, tag="e21")
                nc.vector.tensor_scalar_mul(out=e21[:], in0=pc[:, 0:21], scalar1=d3[:, 0:1])
                tmp21 = cols.tile([128, 21], F32, tag="tmp21")
                nc.vector.tensor_scalar_mul(out=tmp21[:], in0=pc[:, 21:42], scalar1=d3[:, 1:2])
                nc.vector.tensor_tensor(out=e21[:], in0=e21[:], in1=tmp21[:], op=OP.add)
                nc.vector.tensor_scalar_mul(out=tmp21[:], in0=pc[:, 42:63], scalar1=d3[:, 2:3])
                nc.vector.tensor_tensor(out=e21[:], in0=e21[:], in1=tmp21[:], op=OP.add)
                esq = cols.tile([128, 21], F32, tag="esq")
                nc.vector.tensor_tensor(out=esq[:], in0=e21[:], in1=e21[:], op=OP.mult)
                oP = cols.tile([128, 21], F32, tag="oP")
                nc.vector.tensor_scalar_mul(out=oP[:], in0=pc[:, 0:21], scalar1=o3[:, 0:1])
                nc.vector.tensor_scalar_mul(out=tmp21[:], in0=pc[:, 21:42], scalar1=o3[:, 1:2])
                nc.vector.tensor_tensor(out=oP[:], in0=oP[:], in1=tmp21[:], op=OP.add)
                nc.vector.tensor_scalar_mul(out=tmp21[:], in0=pc[:, 42:63], scalar1=o3[:, 2:3])
                nc.vector.tensor_tensor(out=oP[:], in0=oP[:], in1=tmp21[:], op=OP.add)
                oPe = cols.tile([128, 21], F32, tag="oPe")
                nc.vector.tensor_tensor(out=oPe[:], in0=oP[:], in1=e21[:], op=OP.mult)
                oP2 = cols.tile([128, 21], F32, tag="oP2")
                nc.vector.tensor_tensor(out=oP2[:], in0=oP[:], in1=oP[:], op=OP.mult)

                # ---------------- moments ----------------
                tm2a = cols.tile([128, 2 * NS], F32, tag="tm2a")
                tva = cols.tile([128, 2 * NS], F32, tag="tva")
                rva = cols.tile([128, 2 * NS], F32, tag="rva")
                _moments(nc, wide, z2, r2, tm2a, tva, rva, n=2 * NS)
                tm2f, tm2b = tm2a[:, 0:NS], tm2a[:, NS:2 * NS]
                tvf, tvb = tva[:, 0:NS], tva[:, NS:2 * NS]
                rvf, rvb = rva[:, 0:NS], rva[:, NS:2 * NS]

                yvb = yvp.tile([128, NF * NS], F32, tag=f"yv{t}")
                yvbs.append(yvb)
                u0 = u0p.tile([128, NF * NS], I32, tag=f"u0{t}")
                u0s.append(u0)

                # ---------------- fg: cov_diag + angles ----------------
                alf = wide.tile([128, NS], F32, tag="mo_b")
                nc.vector.tensor_scalar_mul(out=alf[:], in0=rvf, scalar1=rdmag[:])
                nc.vector.tensor_tensor(out=alf[:], in0=tvf, in1=alf[:], op=OP.subtract)
                ybf = cols.tile([128, 3 * NS], F32, tag="ybf")   # [s*3+k]
                for k in range(3):
                    # m_k = tm2f * halfd_k + o_k  (compact [s*3+k])
                    nc.vector.tensor_scalar(
                        out=_ap(ybf[:], k, [[3, NS]]), in0=tm2f,
                        scalar1=hd3[:, k:k + 1], scalar2=o3[:, k:k + 1],
                        op0=OP.mult, op1=OP.add)
                    # cd_k = alf * dk2_k + rvf  -> yvb fg col 21+k
                    nc.vector.scalar_tensor_tensor(
                        out=_ap(yvb[:], 21 + k, [[NF, NS]]), in0=alf[:],
                        scalar=dk2[:, k:k + 1], in1=rvf, op0=OP.mult, op1=OP.add)
                # fg angle round chain: f0 = t - round(t), t = yb*inv2pi
                ttf = cols.tile([128, 3 * NS], F32, tag="ttf")
                nc.vector.tensor_scalar(out=ttf[:], in0=ybf[:], scalar1=INV2PI,
                                        scalar2=MAGIC_RND, op0=OP.mult, op1=OP.add)
                nc.vector.tensor_scalar(out=ttf[:], in0=ttf[:], scalar1=MAGIC_RND,
                                        scalar2=None, op0=OP.subtract)
                nc.vector.scalar_tensor_tensor(out=ybf[:], in0=ybf[:], scalar=INV2PI,
                                               in1=ttf[:], op0=OP.mult, op1=OP.subtract)
                # u0 fg cols (strided dst [s*24 + 21+k])
                nc.vector.tensor_scalar_mul(
                    out=_ap(u0[:], 21, [[NF, NS], [1, 3]]), in0=ybf[:],
                    scalar1=float(2.0 ** 32))

                # ---------------- bg: contraction scalars ----------------
                X = base.tile([128, 3 * NS], F32, tag="mk")          # [k*64+s]
                for k in range(3):
                    nc.vector.tensor_scalar(
                        out=X[:, k * NS:(k + 1) * NS], in0=tm2b,
                        scalar1=hd3[:, k:k + 1], scalar2=o3[:, k:k + 1],
                        op0=OP.mult, op1=OP.add)
                s2 = cols.tile([128, NS], F32, tag="s2")
                nc.vector.tensor_tensor(out=s2[:], in0=X[:, 0:NS], in1=X[:, 0:NS], op=OP.mult)
                w0 = wide.tile([128, NS], F32, tag="mo_a")
                nc.vector.tensor_tensor(out=w0[:], in0=X[:, NS:2 * NS], in1=X[:, NS:2 * NS], op=OP.mult)
                nc.vector.tensor_tensor(out=s2[:], in0=s2[:], in1=w0[:], op=OP.add)
                nc.vector.tensor_tensor(out=w0[:], in0=X[:, 2 * NS:3 * NS], in1=X[:, 2 * NS:3 * NS], op=OP.mult)
                nc.vector.tensor_tensor(out=s2[:], in0=s2[:], in1=w0[:], op=OP.add)
                # h = d . X
                h = cols.tile([128, NS], F32, tag="h")
                nc.vector.tensor_scalar_mul(out=h[:], in0=X[:, 0:NS], scalar1=d3[:, 0:1])
                nc.vector.scalar_tensor_tensor(out=h[:], in0=X[:, NS:2 * NS],
                                               scalar=d3[:, 1:2], in1=h[:],
                                               op0=OP.mult, op1=OP.add)
                nc.vector.scalar_tensor_tensor(out=h[:], in0=X[:, 2 * NS:3 * NS],
                                               scalar=d3[:, 2:3], in1=h[:],
                                               op0=OP.mult, op1=OP.add)

                # rsqrt(s2): magic seed + 3 Newton iterations
                rn0 = cols.tile([128, NS], F32, tag="rn0")
                seed_u = wide.tile([128, NS], U32, tag="mo_a")
                nc.vector.tensor_scalar(out=seed_u[:], in0=s2[:].bitcast(U32),
                                        scalar1=1, scalar2=None,
                                        op0=OP.logical_shift_right)
                nc.vector.tensor_tensor(
                    out=rn0[:].bitcast(U32),
                    in0=_ap(magic_u[:], 0, [[0, NS]]),
                    in1=seed_u[:], op=OP.subtract)
                for _ in range(3):
                    nr = wide.tile([128, NS], F32, tag="mo_b")
                    nc.vector.tensor_tensor(out=nr[:], in0=s2[:], in1=rn0[:], op=OP.mult)
                    nc.vector.tensor_tensor(out=nr[:], in0=nr[:], in1=rn0[:], op=OP.mult)
                    nc.vector.tensor_scalar(out=nr[:], in0=nr[:], scalar1=-0.5,
                                            scalar2=1.5, op0=OP.mult, op1=OP.add)
                    nc.vector.tensor_tensor(out=rn0[:], in0=rn0[:], in1=nr[:], op=OP.mult)

                n0 = cols.tile([128, NS], F32, tag="n0")
                nc.vector.tensor_tensor(out=n0[:], in0=s2[:], in1=rn0[:], op=OP.mult)
                rn = cols.tile([128, NS], F32, tag="rn")
                nc.vector.tensor_scalar(out=rn[:], in0=rn0[:], scalar1=-TINY,
                                        scalar2=1.0, op0=OP.mult, op1=OP.add)
                nc.vector.tensor_tensor(out=rn[:], in0=rn0[:], in1=rn[:], op=OP.mult)
                a_ = cols.tile([128, NS], F32, tag="a")
                nc.vector.tensor_scalar(out=a_[:], in0=rn[:], scalar1=-1.0,
                                        scalar2=2.0, op0=OP.mult, op1=OP.add)
                nc.vector.tensor_tensor(out=a_[:], in0=rn[:], in1=a_[:], op=OP.mult)
                b_ = cols.tile([128, NS], F32, tag="b")
                nc.vector.tensor_scalar_add(out=b_[:], in0=rn[:], scalar1=-1.0)
                t2_ = wide.tile([128, NS], F32, tag="mo_a")
                nc.vector.tensor_tensor(out=t2_[:], in0=rn[:], in1=rn0[:], op=OP.mult)
                nc.vector.tensor_tensor(out=t2_[:], in0=t2_[:], in1=rn[:], op=OP.mult)
                nc.vector.tensor_tensor(out=b_[:], in0=t2_[:], in1=b_[:], op=OP.mult)
                nc.vector.tensor_scalar_mul(out=b_[:], in0=b_[:], scalar1=2.0)

                # alpha_b, A coefficients
                alb = cols.tile([128, NS], F32, tag="alb")
                nc.vector.tensor_scalar_mul(out=alb[:], in0=rvb, scalar1=rdmag[:])
                nc.vector.tensor_tensor(out=alb[:], in0=tvb, in1=alb[:], op=OP.subtract)
                bh = cols.tile([128, NS], F32, tag="bh")
                nc.vector.tensor_tensor(out=bh[:], in0=b_[:], in1=h[:], op=OP.mult)
                asq = wide.tile([128, NS], F32, tag="mo_a")
                nc.vector.tensor_tensor(out=asq[:], in0=a_[:], in1=a_[:], op=OP.mult)
                A1 = cols.tile([128, NS], F32, tag="A1")
                nc.vector.tensor_tensor(out=A1[:], in0=alb[:], in1=asq[:], op=OP.mult)
                A4 = cols.tile([128, NS], F32, tag="A4")
                nc.vector.tensor_tensor(out=A4[:], in0=rvb, in1=asq[:], op=OP.mult)
                A2 = cols.tile([128, NS], F32, tag="A2")
                nc.vector.tensor_tensor(out=A2[:], in0=alb[:], in1=a_[:], op=OP.mult)
                nc.vector.tensor_tensor(out=A2[:], in0=A2[:], in1=bh[:], op=OP.mult)
                nc.vector.tensor_scalar_mul(out=A2[:], in0=A2[:], scalar1=2.0)
                # A3 = alb*bh^2 + rvb*(2ab + (b*n0)^2)
                A3 = cols.tile([128, NS], F32, tag="A3")
                bn = wide.tile([128, NS], F32, tag="mo_b")
                nc.vector.tensor_tensor(out=bn[:], in0=b_[:], in1=n0[:], op=OP.mult)
                nc.vector.tensor_tensor(out=bn[:], in0=bn[:], in1=bn[:], op=OP.mult)
                ab = wide.tile([128, NS], F32, tag="mo_c")
                nc.vector.tensor_tensor(out=ab[:], in0=a_[:], in1=b_[:], op=OP.mult)
                nc.vector.scalar_tensor_tensor(out=bn[:], in0=ab[:], scalar=2.0,
                                               in1=bn[:], op0=OP.mult, op1=OP.add)
                nc.vector.tensor_tensor(out=A3[:], in0=rvb, in1=bn[:], op=OP.mult)
                bh2 = wide.tile([128, NS], F32, tag="mo_a")
                nc.vector.tensor_tensor(out=bh2[:], in0=bh[:], in1=bh[:], op=OP.mult)
                nc.vector.tensor_tensor(out=bh2[:], in0=alb[:], in1=bh2[:], op=OP.mult)
                nc.vector.tensor_tensor(out=A3[:], in0=A3[:], in1=bh2[:], op=OP.add)

                # ---- rank-4 coefficients: yv = C1*esq + C2*oPe + A3*oP2 + A4*w
                # tm = t_mean;  C1 = A1 + tm*(A2 + tm*A3);  C2 = A2 + 2*tm*A3
                tm = cols.tile([128, NS], F32, tag="tm")
                nc.vector.tensor_scalar_mul(out=tm[:], in0=tm2b, scalar1=0.5)
                v_ = wide.tile([128, NS], F32, tag="mo_b")
                nc.vector.tensor_tensor(out=v_[:], in0=tm[:], in1=A3[:], op=OP.mult)
                C2 = cols.tile([128, NS], F32, tag="C2")
                nc.vector.scalar_tensor_tensor(out=C2[:], in0=v_[:], scalar=2.0,
                                               in1=A2[:], op0=OP.mult, op1=OP.add)
                C1 = cols.tile([128, NS], F32, tag="C1")
                nc.vector.tensor_tensor(out=C1[:], in0=v_[:], in1=A2[:], op=OP.add)
                nc.vector.tensor_tensor(out=C1[:], in0=tm[:], in1=C1[:], op=OP.mult)
                nc.vector.tensor_tensor(out=C1[:], in0=C1[:], in1=A1[:], op=OP.add)
                # B1 = a/2pi, B2 = a*tm/2pi  (angle-in-turns coefficients)
                B1 = cols.tile([128, NS], F32, tag="B1")
                nc.vector.tensor_scalar_mul(out=B1[:], in0=a_[:], scalar1=INV2PI)
                B2 = cols.tile([128, NS], F32, tag="B2")
                nc.vector.tensor_tensor(out=B2[:], in0=B1[:], in1=tm[:], op=OP.mult)

                # ---- bg yv: 7 broadcast TTs into yvb strided [s*24+f], f<21
                yvb_bg = _ap(yvb[:], 0, [[NF, NS], [1, 21]])
                w1 = base.tile([128, 21 * NS], F32, tag="w1")
                C1b = _ap(C1[:], 0, [[1, NS], [0, 21]])
                C2b = _ap(C2[:], 0, [[1, NS], [0, 21]])
                A3b = _ap(A3[:], 0, [[1, NS], [0, 21]])
                A4b = _ap(A4[:], 0, [[1, NS], [0, 21]])
                esq_b = _ap(esq[:], 0, [[0, NS], [1, 21]])
                oPe_b = _ap(oPe[:], 0, [[0, NS], [1, 21]])
                oP2_b = _ap(oP2[:], 0, [[0, NS], [1, 21]])
                w_b = _ap(pc[:], 63, [[0, NS], [1, 21]])
                nc.vector.tensor_tensor(out=yvb_bg, in0=C1b, in1=esq_b, op=OP.mult)
                nc.vector.tensor_tensor(out=w1[:], in0=C2b, in1=oPe_b, op=OP.mult)
                nc.vector.tensor_tensor(out=yvb_bg, in0=yvb_bg, in1=w1[:], op=OP.add)
                nc.vector.tensor_tensor(out=w1[:], in0=A3b, in1=oP2_b, op=OP.mult)
                nc.vector.tensor_tensor(out=yvb_bg, in0=yvb_bg, in1=w1[:], op=OP.add)
                nc.vector.tensor_tensor(out=w1[:], in0=A4b, in1=w_b, op=OP.mult)
                nc.vector.tensor_tensor(out=yvb_bg, in0=yvb_bg, in1=w1[:], op=OP.add)

                # ---- bg angles pre-scaled: y' = B1*oP + B2*e  ([s*21+q])
                B1b = _ap(B1[:], 0, [[1, NS], [0, 21]])
                B2b = _ap(B2[:], 0, [[1, NS], [0, 21]])
                oP_b = _ap(oP[:], 0, [[0, NS], [1, 21]])
                e_b = _ap(e21[:], 0, [[0, NS], [1, 21]])
                y1 = base.tile([128, 21 * NS], F32, tag="y1")
                nc.vector.tensor_tensor(out=y1[:], in0=B2b, in1=e_b, op=OP.mult)
                nc.vector.tensor_tensor(out=w1[:], in0=B1b, in1=oP_b, op=OP.mult)
                nc.vector.tensor_tensor(out=y1[:], in0=y1[:], in1=w1[:], op=OP.add)
                # range-reduce (|y1| can exceed 0.5 turns when the contraction
                # blows up near the origin): f0 = y1 - round(y1)
                nc.vector.tensor_scalar(out=w1[:], in0=y1[:], scalar1=MAGIC_RND,
                                        scalar2=None, op0=OP.add)
                nc.vector.tensor_scalar(out=w1[:], in0=w1[:], scalar1=MAGIC_RND,
                                        scalar2=None, op0=OP.subtract)
                nc.vector.tensor_tensor(out=y1[:], in0=y1[:], in1=w1[:],
                                        op=OP.subtract)
                # u0 bg cols: f32 -> i32 convert of the turn fraction
                nc.vector.tensor_scalar_mul(
                    out=_ap(u0[:], 0, [[NF, NS], [1, 21]]), in0=y1[:],
                    scalar1=float(2.0 ** 32))

            # ---------------- streaming: per-eighth pipeline ----------
            # E, S, D are stored s-major: [s*384 + j*24 + f] (strided ACT
            # dst is free; makes every product operand <=3D and contiguous
            # per sample, which walrus' stt verifier requires).
            def emit_exp(E, t, hh, j0, j1):
                for j in range(j0, j1):
                    nc.scalar.activation(
                        out=_ap(E[:], j * NF, [[NL * NF, HALF], [1, NF]]),
                        in_=yvbs[t][:, hh * NF * HALF:(hh + 1) * NF * HALF],
                        func=AF.Exp, scale=float(-0.5 * (4.0 ** j)))

            def cascade(t, e_idx):
                """i32 shift cascade for one eighth -> us [j*W + s*24+f]."""
                u0 = u0s[t]
                us = usp.tile([128, NL * W], I32, tag="us")
                nc.vector.tensor_copy(out=us[:, 0:W],
                                      in_=u0[:, e_idx * W:(e_idx + 1) * W])
                for b, sh in ((1, 1), (2, 2), (4, 4), (8, 8)):
                    nc.vector.tensor_scalar(
                        out=us[:, b * W:2 * b * W], in0=us[:, 0:b * W],
                        scalar1=sh, scalar2=None,
                        op0=OP.logical_shift_left)
                return us

            ang = {}
            ecur = ep.tile([128, NL * NF * HALF], F16, tag="E")
            emit_exp(ecur, 0, 0, 0, NL)     # prologue: E for (t=0, h=0)
            for t in range(2):
                ang[t] = cascade(t, 0)
                for hh in range(2):
                    nt, nh = (t, 1) if hh == 0 else (t + 1, 0)
                    enext = None
                    if nt < 2:
                        enext = ep.tile([128, NL * NF * HALF], F16, tag="E")

                    for ee in range(4):
                        e_idx = hh * 4 + ee          # eighth within tile
                        r0 = t * 128
                        E = ecur
                        us = ang.pop(t)

                        # --- ScE: S2 holds 17 levels per sample [s*408 + (j+1)*24+f]
                        # level -1 (cols 0:24 per s) is the half-angle sin;
                        # levels 0..15 (cols 24:408 per s) are sin(theta_j).
                        # One Square ACT over levels -1..14 then yields
                        # sq[s, j] = sin^2(theta_j / 2) for all 16 levels.
                        SW = (NL + 1) * NF           # 408
                        S2 = sp.tile([128, EI * SW], F16, tag="S")
                        nc.scalar.activation(
                            out=_ap(S2[:], 0, [[SW, EI], [1, NF]]),
                            in_=us[:, 0:W], func=AF.Arctan,
                            scale=float(2.0 ** -33))
                        nc.scalar.activation(
                            out=_ap(S2[:], NF, [[NF, NL], [SW, EI], [1, NF]]),
                            in_=us[:], func=AF.Arctan,
                            scale=float(2.0 ** -32))
                        sq = sp.tile([128, NL * W], F16, tag="sq")
                        nc.scalar.activation(
                            out=_ap(sq[:], 0, [[384, EI], [1, 384]]),
                            in_=_ap(S2[:], 0, [[SW, EI], [1, 384]]),
                            func=AF.Square)

                        # --- software pipeline: next eighth's cascade (DVE)
                        if e_idx < 7:
                            ang[t] = cascade(t, e_idx + 1)

                        # --- spread next half's exp instrs (4 per eighth) ---
                        if enext is not None:
                            emit_exp(enext, nt, nh, 4 * ee, 4 * ee + 4)

                        # --- C = 1 - 2*sq  (4x TS on DVE) ---
                        C = dp.tile([128, NL * W], F16, tag="C")
                        nc.vector.tensor_scalar(out=C[:], in0=sq[:],
                                                scalar1=-2.0, scalar2=1.0,
                                                op0=OP.mult, op1=OP.add)

                        # --- products into interleaved s-major out tile ---
                        ob = outp.tile([128, EI * FOUT], F16, tag="ob")
                        e_off = ee * EI * NL * NF   # E offset for this eighth
                        s_src = _ap(S2[:], NF, [[SW, EI], [1, 384]])
                        c_src = _ap(C[:], 0, [[NL * NF, EI], [1, 384]])
                        e_src = _ap(E[:], e_off, [[NL * NF, EI], [1, 384]])
                        o_sin = _ap(ob[:], 0, [[FOUT, EI], [1, 384]])
                        o_cos = _ap(ob[:], 384, [[FOUT, EI], [1, 384]])
                        nc.vector.tensor_tensor(out=o_sin, in0=s_src,
                                                in1=e_src, op=OP.mult)
                        nc.vector.tensor_tensor(out=o_cos, in0=c_src,
                                                in1=e_src, op=OP.mult)

                        # --- DMA out (single contiguous block) ---
                        oa = out[:, :]
                        nc.sync.dma_start(
                            out=bass.AP(
                                tensor=oa.tensor,
                                offset=oa.offset + r0 * NS * FOUT + e_idx * EI * FOUT,
                                ap=[[NS * FOUT, 128], [1, EI * FOUT]]),
                            in_=ob[:])
                    ecur = enext

    _split_sync_waits(nc)
    return nc


# ---------------------------------------------------------------------------
# entry point
# ---------------------------------------------------------------------------

_NC_CACHE = []


def kernel(ray_o, ray_d, fg_z_vals, bg_z_vals, radii):
    from concourse.bass_utils import run_bass_kernel_spmd

    if not _NC_CACHE:
        _NC_CACHE.append(build_kernel())
    nc = _NC_CACHE[0]

    pconst = np.concatenate(
        [P_BASIS.reshape(-1), (P_BASIS * P_BASIS).sum(axis=0)]).astype(np.float32)[None, :]

    in_maps = []
    for cidx in range(N_CORES):
        sl = slice(cidx * RAYS_PER_CORE, (cidx + 1) * RAYS_PER_CORE)
        in_maps.append({
            "ray_o": np.ascontiguousarray(ray_o[sl]).astype(np.float32, copy=False),
            "ray_d": np.ascontiguousarray(ray_d[sl]).astype(np.float32, copy=False),
            "fg_z": np.ascontiguousarray(fg_z_vals[sl]).astype(np.float32, copy=False),
            "bg_z": np.ascontiguousarray(bg_z_vals[sl]).astype(np.float32, copy=False),
            "radii": np.ascontiguousarray(radii[sl]).astype(np.float32, copy=False),
            "pconst": pconst,
        })

    res = run_bass_kernel_spmd(nc, in_maps, core_ids=list(range(N_CORES)))
    outs = [res.results[i]["out"].reshape(RAYS_PER_CORE, NS, FOUT)
            for i in range(N_CORES)]
    raw = np.concatenate(outs, axis=0)
    return raw[..., OUT_PERM].astype(np.float32)
